# revision 12
# baseline (speedup 1.0000x reference)
"""CertViT (ViT-Base + layer-3 token pruning) forward pass on 8 Trainium2 cores.

Data parallel: 8 images per core, processed as 4 image-pairs so dense matmul
free dims (394 / 278) stay >= 256. Activations live in channel-partition
layout x^T [768 -> 6x128 chunks, tokens]; the residual stream x stays fp32,
everything fed to the PE (post-LN activations, q/k/v, exp weights, weights)
is fp16 so small-free-dim attention matmuls run at 1 cycle/row and DVE ops
get the 2x/4x modes. Attention processes heads in even/odd pairs: QK is
row-tiled (contraction 64: even head rows 0:63, odd 64:127), the softmax
denominator and AV are col-tiled (output partitions 0:63 / 64:127), so head
pairs run concurrently in the PE array and odd heads no longer need a
partition-shift DMA. Both images of a pair share one QK PSUM bank
([keys, img0 queries | img1 queries]), halving exp instruction count and AV
streamed columns. All reciprocals (softmax denom, LN rsqrt, uncertainty) are
computed as Exp(-k*Ln(x)) on ScalarE -- one activation-table set shared with
the attention Exp, nothing iterative on DVE. LayerNorm affine params are
folded into the following matmul weights on the host. Top-k pruning uses
max8/match_replace for the drop mask, a triangular-matmul cumsum for ranks,
and a one-hot permutation matmul for the gather.
"""

import os
import sys

import numpy as np

for _p in ('/opt/trn_rl_repo', '/root/.axon_site/_ro/trn_rl_repo'):
    if os.path.isdir(_p) and _p not in sys.path:
        sys.path.append(_p)

import concourse.bass as bass
import concourse.mybir as mybir
from concourse.tile import TileContext
from concourse.bass_utils import run_bass_kernel_spmd
from concourse.alu_op_type import AluOpType as AL

dt = mybir.dt
AF = mybir.ActivationFunctionType

# ---------------------------------------------------------------- config
NCORES = 8
B_CORE = 8            # images per core
PAIRS = B_CORE // 2
C = 768
CH = C // 128          # 6 channel chunks
HD = 12                # heads
HP = HD // 2           # head pairs
D = 64                 # head dim
SCALE = D ** -0.5
DEPTH = 12
SEL = 3                # pruning layer
N0 = 197               # tokens before pruning
K_KEEP = 137           # int(197*0.7)
N_DROP = N0 - 1 - K_KEEP   # 59
N1 = K_KEEP + 2        # 139 tokens after pruning
F0 = 2 * N0            # pair free dim, layers 0..3
F1 = 2 * N1            # pair free dim, layers 4..11
EPS = 1e-6
NCLS = 100

# ------------------------------------------------------------- waitfix
# This walrus build accepts at most ONE sem wait per instruction; Tile can
# attach several. Move excess waits onto InstNoOp carriers inserted before.
_wf_counter = [0]


def _wf_carrier(engine, waits):
    _wf_counter[0] += 1
    d = mybir.InstNoOp(name=f"waitfix-{_wf_counter[0]}", ins=[], outs=[])
    d.engine = engine
    d.sync_info = mybir.SyncInfo(on_wait=list(waits), on_update=[])
    return d


def split_excess_waits(nc, max_waits=1):
    nfix = 0
    for f in nc.m.functions:
        for bb in f.blocks:
            insts = list(bb.instructions)
            out = []
            changed = False
            for inst in insts:
                si = inst.sync_info
                waits = list(si.on_wait) if si and si.on_wait else []
                if len(waits) > max_waits:
                    keep, rest = waits[:max_waits], waits[max_waits:]
                    while rest:
                        chunk, rest = rest[:max_waits], rest[max_waits:]
                        out.append(_wf_carrier(inst.engine, chunk))
                    si.on_wait = keep
                    changed = True
                    nfix += 1
                out.append(inst)
            if changed:
                bb.instructions = out
    return nfix


# ----------------------------------------------------------- device kernel
def build_nc():
    nc = bass.Bass()
    f32, f32r, f16 = dt.float32, dt.float32r, dt.float16

    d = {}
    d["patches_d"] = nc.declare_dram_parameter("patchesT", [C, B_CORE * 196], f32r, isOutput=False)
    d["posc_d"] = nc.declare_dram_parameter("posCT", [C, N0], f32, isOutput=False)
    d["pw_d"] = nc.declare_dram_parameter("patch_wT", [C, C], f32r, isOutput=False)
    d["qkvw_d"] = nc.declare_dram_parameter("qkv_wT", [DEPTH, C, 3 * C], f16, isOutput=False)
    d["qkvb_d"] = nc.declare_dram_parameter("qkv_bL", [DEPTH, 128, 18], f32, isOutput=False)
    d["projw_d"] = nc.declare_dram_parameter("proj_wT", [DEPTH, C, C], f16, isOutput=False)
    d["projb_d"] = nc.declare_dram_parameter("proj_bL", [DEPTH, 128, 6], f32, isOutput=False)
    d["fc1w_d"] = nc.declare_dram_parameter("fc1_wT", [DEPTH, C, 4 * C], f16, isOutput=False)
    d["fc1b_d"] = nc.declare_dram_parameter("fc1_bL", [DEPTH, 128, 24], f32, isOutput=False)
    d["fc2w_d"] = nc.declare_dram_parameter("fc2_wT", [DEPTH, 4 * C, C], f16, isOutput=False)
    d["fc2b_d"] = nc.declare_dram_parameter("fc2_bL", [DEPTH, 128, 6], f32, isOutput=False)
    d["headw_d"] = nc.declare_dram_parameter("headT", [C, NCLS], f16, isOutput=False)
    d["headb_d"] = nc.declare_dram_parameter("head_bL", [NCLS, 1], f32, isOutput=False)
    d["ident_d"] = nc.declare_dram_parameter("ident", [128, 128], f32, isOutput=False)
    d["ones_d"] = nc.declare_dram_parameter("ones", [128, 128], f16, isOutput=False)
    d["invc_d"] = nc.declare_dram_parameter("invC", [128, 128], f32r, isOutput=False)
    d["iota_d"] = nc.declare_dram_parameter("iota", [128, N1 - 1], f32, isOutput=False)
    d["lt_d"] = nc.declare_dram_parameter("LT", [196, 196], f32r, isOutput=False)
    d["out_d"] = nc.declare_dram_parameter("logitsT", [NCLS, B_CORE], f32, isOutput=True)

    d["dbg_layer"] = os.environ.get("BASS_VIT_DEBUG_LAYER", "")
    if d["dbg_layer"]:
        d["dbg_d"] = nc.declare_dram_parameter("dbg", [1 + 2 * DEPTH, 128, CH * F0], f32, isOutput=True)
        d["dbgp_d"] = nc.declare_dram_parameter("dbgp", [4, 8, 196], f32, isOutput=True)
    else:
        d["dbg_d"] = None
        d["dbgp_d"] = None

    with TileContext(nc) as tc:
        _build_body(nc, tc, d)
    return nc


def _build_body(nc, tc, d):
    f32, f32r, f16 = dt.float32, dt.float32r, dt.float16
    from contextlib import ExitStack
    es = ExitStack()

    cpool = es.enter_context(tc.tile_pool(name="consts", bufs=1))
    xpool = es.enter_context(tc.tile_pool(name="x", bufs=1))
    ppool = es.enter_context(tc.tile_pool(name="psum", bufs=1, space="PSUM"))
    prpool = es.enter_context(tc.tile_pool(name="prune", bufs=1))
    bpool = es.enter_context(tc.tile_pool(name="bias", bufs=2))

    # constants
    ident = cpool.tile([128, 128], f32, tag="ident")
    ones = cpool.tile([128, 128], f16, tag="ones")
    invc = cpool.tile([128, 128], f32r, tag="invc")
    iota = cpool.tile([128, N1 - 1], f32, tag="iota")
    ltt = cpool.tile([128, 2 * 196], f32r, tag="ltt")
    posct = cpool.tile([128, CH * N0], f32, tag="posct")
    eps_t = cpool.tile([128, 1], f32, tag="eps_t")
    nc.vector.memset(eps_t[:], EPS)
    n0_t = cpool.tile([128, 1], f32, tag="n0_t")
    nc.vector.memset(n0_t[:], float(N0))
    nc.sync.dma_start(ident[:], d["ident_d"][:])
    nc.sync.dma_start(ones[:], d["ones_d"][:])
    nc.sync.dma_start(invc[:], d["invc_d"][:])
    nc.sync.dma_start(iota[:], d["iota_d"][:])
    nc.sync.dma_start(ltt[:, 0:196], d["lt_d"][0:128, :])
    nc.sync.dma_start(ltt[0:68, 196:392], d["lt_d"][128:196, :])
    nc.sync.dma_start(posct[:].rearrange("p (k n) -> p k n", k=CH), d["posc_d"].rearrange("(k p) n -> p k n", p=128))

    # PSUM slots: tag 'a' x4 (main accumulations + QK), 'b' x2 (denominator),
    # 'c' x2 (AV / LN meansq) -> 8 banks
    def psA():
        return ppool.tile([128, F0], f32, tag="a", bufs=4, name="psA")

    def psB():
        return ppool.tile([128, F0], f32, tag="b", bufs=2, name="psB")

    def psC():
        return ppool.tile([128, F0], f32, tag="c", bufs=2, name="psC")

    # persistent per-pair residual stream x^T, chunk-major [128, CH*F]
    xt = [xpool.tile([128, CH * F0], f32r, tag=f"x{p}", name=f"x{p}") for p in range(PAIRS)]
    # per-pair uncertainty rows (filled at layer SEL)
    unc = [prpool.tile([1, F0], f32, tag=f"unc{p}", name=f"unc{p}") for p in range(PAIRS)]

    # ------------------------------------------------------------ patch embed
    with tc.tile_pool(name="wpatch", bufs=1) as wp, tc.tile_pool(name="tpatch", bufs=2) as tp:
        pwt = wp.tile([128, CH * C], f32r, tag="pw")
        nc.sync.dma_start(pwt[:].rearrange("p (k n) -> p k n", k=CH), d["pw_d"].rearrange("(k p) n -> p k n", p=128))
        for p in range(PAIRS):
            prt = tp.tile([128, CH * 392], f32r, tag="patches")
            nc.sync.dma_start(
                prt[:].rearrange("p (k n) -> p k n", k=CH),
                d["patches_d"][:, p * 392:(p + 1) * 392].rearrange("(k p) n -> p k n", p=128),
            )
            for co in range(CH):
                ps = psA()
                for k in range(CH):
                    nc.tensor.matmul(
                        ps[:, 0:392],
                        pwt[:, k * C + co * 128: k * C + co * 128 + 128],
                        prt[:, k * 392:(k + 1) * 392],
                        start=(k == 0), stop=(k == CH - 1),
                    )
                for b in range(2):
                    nc.vector.tensor_tensor(
                        xt[p][:, co * F0 + b * N0 + 1: co * F0 + b * N0 + N0],
                        ps[:, b * 196:(b + 1) * 196],
                        posct[:, co * N0 + 1: co * N0 + N0],
                        op=AL.add,
                    )
                    nc.vector.tensor_copy(
                        xt[p][:, co * F0 + b * N0: co * F0 + b * N0 + 1],
                        posct[:, co * N0: co * N0 + 1],
                    )

    def tap(slot, xtile, F):
        if d["dbg_d"] is not None:
            nc.sync.dma_start(d["dbg_d"][slot][:, 0:CH * F], xtile[:, 0:CH * F].bitcast(f32))

    tap(0, xt[0], F0)

    # ------------------------------------------------------------ helpers
    def layernorm(pool, x, F, xh_tag, xh_bufs=1):
        """Standardize x (chunk-major [128, CH*F]) per token -> fp16 tile."""
        xh = pool.tile([128, CH * F], f16, tag=xh_tag, bufs=xh_bufs, name=xh_tag)
        sq = pool.tile([128, CH * F], f32r, tag="ln_sq", bufs=1)
        for k in range(CH):
            nc.vector.tensor_tensor(
                sq[:, k * F:(k + 1) * F],
                x[:, k * F:(k + 1) * F].bitcast(f32),
                x[:, k * F:(k + 1) * F].bitcast(f32),
                op=AL.mult,
            )
        pm = psB()
        ps2 = psC()
        for k in range(CH):
            nc.tensor.matmul(pm[:, 0:F], invc[:], x[:, k * F:(k + 1) * F],
                             start=(k == 0), stop=(k == CH - 1))
        for k in range(CH):
            nc.tensor.matmul(ps2[:, 0:F], invc[:], sq[:, k * F:(k + 1) * F],
                             start=(k == 0), stop=(k == CH - 1))
        var = pool.tile([128, F], f32, tag="ln_var", bufs=1)
        rstd = pool.tile([128, F], f32, tag="ln_rstd", bufs=1)
        mean = pool.tile([128, F], f32, tag="ln_mean", bufs=1)
        nc.vector.tensor_copy(mean[:], pm[:, 0:F])
        nc.vector.tensor_tensor(var[:], mean[:], mean[:], op=AL.mult)
        nc.vector.tensor_tensor(var[:], ps2[:, 0:F], var[:], op=AL.subtract)
        # rstd = exp(-0.5*ln(var+eps)) = 1/sqrt(var+eps); Ln+Exp share one
        # activation-table set with the attention Exp.
        nc.scalar.activation(rstd[:], var[:], AF.Ln, bias=eps_t[:, 0:1])
        nc.scalar.activation(rstd[:], rstd[:], AF.Exp, scale=-0.5)
        for k in range(CH):
            nc.vector.tensor_tensor(
                var[:], x[:, k * F:(k + 1) * F].bitcast(f32), mean[:], op=AL.subtract)
            nc.vector.tensor_tensor(
                xh[:, k * F:(k + 1) * F], var[:], rstd[:], op=AL.mult)
        return xh

    def load_bias(dram_t, l, cols):
        bt = bpool.tile([128, cols], f32, tag=dram_t.name)
        nc.sync.dma_start(bt[:], dram_t[l])
        return bt

    # ------------------------------------------------------------ layers
    for l in range(DEPTH):
        F = F0 if l <= SEL else F1
        N = N0 if l <= SEL else N1
        mlens = [128, N - 128]

        qkvb = load_bias(d["qkvb_d"], l, 18)
        projb = load_bias(d["projb_d"], l, 6)

        # ---------------- phase A: LN1 + QKV + attention + proj ----------------
        with tc.tile_pool(name="wA", bufs=1) as wA, tc.tile_pool(name="tA", bufs=1) as tA:
            wq = wA.tile([128, CH * 3 * C], f16, tag="wqkv")
            nc.sync.dma_start(wq[:].rearrange("p (k n) -> p k n", k=CH), d["qkvw_d"][l].rearrange("(k p) n -> p k n", p=128))
            wpj = wA.tile([128, CH * C], f16, tag="wproj")
            nc.sync.dma_start(wpj[:].rearrange("p (k n) -> p k n", k=CH), d["projw_d"][l].rearrange("(k p) n -> p k n", p=128))

            for p in range(PAIRS):
                xh = layernorm(tA, xt[p], F, "ln1", xh_bufs=2)
                qT = tA.tile([128, CH * F], f16, tag="qT", bufs=2, name="qT")
                kT = tA.tile([128, CH * F], f16, tag="kT", bufs=2, name="kT")
                for o in range(12):
                    ps = psA()
                    for k in range(CH):
                        nc.tensor.matmul(
                            ps[:, 0:F],
                            wq[:, k * 3 * C + o * 128: k * 3 * C + o * 128 + 128],
                            xh[:, k * F:(k + 1) * F],
                            start=(k == 0), stop=(k == CH - 1),
                        )
                    oc = o % CH
                    if o < CH:
                        nc.vector.tensor_scalar(
                            qT[:, oc * F:(oc + 1) * F], ps[:, 0:F],
                            qkvb[:, o:o + 1], SCALE, op0=AL.add, op1=AL.mult)
                    else:
                        nc.vector.tensor_scalar(
                            kT[:, oc * F:(oc + 1) * F], ps[:, 0:F],
                            qkvb[:, o:o + 1], None, op0=AL.add)

                # v in token-partition layout, per image: 2 t-chunks
                vto = [[None, None], [None, None]]
                for b in range(2):
                    for tchunk in range(2):
                        tlen = mlens[tchunk]
                        toff = b * N + tchunk * 128
                        vt = tA.tile([128, C], f16, tag=f"v{b}{tchunk}", bufs=2)
                        vto[b][tchunk] = vt
                        for half in range(2):
                            ps = psA()
                            for k in range(CH):
                                nc.tensor.matmul(
                                    ps[0:tlen, 0:384],
                                    xh[:, k * F + toff: k * F + toff + tlen],
                                    wq[:, k * 3 * C + 2 * C + half * 384:
                                       k * 3 * C + 2 * C + half * 384 + 384],
                                    start=(k == 0), stop=(k == CH - 1),
                                )
                            nc.vector.tensor_copy(
                                vt[0:tlen, half * 384:(half + 1) * 384],
                                ps[0:tlen, 0:384])

                # attention by head pair hp: even head e=0 on rows/out-cols
                # 0:64, odd e=1 on 64:128 (row-tiled QK, col-tiled denom/AV).
                oT = tA.tile([128, CH * F], f16, tag="oT", bufs=2, name="oT")
                for hp in range(HP):
                    qcol = hp * F
                    et = [[tA.tile([128, F], f16, tag=f"et{e}{t}", bufs=2,
                                   name=f"et{e}{t}") for t in range(2)]
                          for e in range(2)]
                    pden = psB()
                    # one AV bank per image; parities col-tiled on disjoint
                    # partitions (0:64 / 64:128), so their accumulation groups
                    # interleave safely within a bank.
                    pav = [psC(), psC()]
                    pev = [psB(), psA()] if l == SEL else None
                    for tchunk in range(2):
                        tlen = mlens[tchunk]
                        toff = tchunk * 128
                        psQK = [psA(), psA()]
                        for b in range(2):
                            for e in range(2):
                                nc.tensor.matmul(
                                    psQK[e][0:tlen, b * N:(b + 1) * N],
                                    kT[e * 64:e * 64 + 64,
                                       qcol + b * N + toff: qcol + b * N + toff + tlen],
                                    qT[e * 64:e * 64 + 64, qcol + b * N: qcol + (b + 1) * N],
                                    start=True, stop=True,
                                )
                        for e in range(2):
                            nc.scalar.activation(
                                et[e][tchunk][0:tlen, 0:F],
                                psQK[e][0:tlen, 0:F], AF.Exp)
                        if l == SEL:
                            for e in range(2):
                                rt = tA.tile([128, F], f16, tag=f"relu{e}", bufs=1)
                                nc.vector.tensor_scalar(
                                    rt[0:tlen, 0:F], psQK[e][0:tlen, 0:F],
                                    0.0, None, op0=AL.max)
                                nc.tensor.matmul(
                                    pev[e][0:1, 0:F], ones[0:tlen, 0:1],
                                    rt[0:tlen, 0:F],
                                    start=(tchunk == 0), stop=(tchunk == 1),
                                )
                    # Denominator + AV after the QK/exp of both tchunks.
                    # A start=True clears the has_written bits for the touched
                    # partitions' whole bank row, so accumulation groups that
                    # share partitions in one bank must never interleave; all
                    # groups below share a bank only across disjoint partition
                    # ranges (col-tiled parities), which is safe.
                    for tchunk in range(2):
                        tlen = mlens[tchunk]
                        for e in range(2):
                            nc.tensor.matmul(
                                pden[e * 64:e * 64 + 64, 0:F],
                                ones[0:tlen, 0:64],
                                et[e][tchunk][0:tlen, 0:F],
                                start=(tchunk == 0), stop=(tchunk == 1),
                            )
                        for b in range(2):
                            for e in range(2):
                                nc.tensor.matmul(
                                    pav[b][e * 64:e * 64 + 64, 0:N],
                                    vto[b][tchunk][0:tlen,
                                                   (2 * hp + e) * 64:(2 * hp + e) * 64 + 64],
                                    et[e][tchunk][0:tlen, b * N:(b + 1) * N],
                                    start=(tchunk == 0), stop=(tchunk == 1),
                                )
                    if l == SEL:
                        # unc += 1/(evidence_sum + N) per head
                        for e in range(2):
                            ev1 = tA.tile([1, F], f32, tag="ev1", bufs=2)
                            nc.scalar.activation(
                                ev1[:], pev[e][0:1, 0:F], AF.Ln,
                                bias=n0_t[0:1, 0:1])
                            nc.scalar.activation(ev1[:], ev1[:], AF.Exp, scale=-1.0)
                            if hp == 0 and e == 0:
                                nc.vector.tensor_copy(unc[p][:], ev1[:])
                            else:
                                nc.vector.tensor_tensor(
                                    unc[p][:], ev1[:], unc[p][:], op=AL.add)
                    # rsb = 1/denominator via exp(-ln), both parities at once
                    rsb = tA.tile([128, F], f32, tag="rsb", bufs=2)
                    nc.scalar.activation(rsb[:], pden[0:128, 0:F], AF.Ln)
                    nc.scalar.activation(rsb[:], rsb[:], AF.Exp, scale=-1.0)
                    for b in range(2):
                        nc.vector.tensor_tensor(
                            oT[:, qcol + b * N:qcol + (b + 1) * N],
                            pav[b][0:128, 0:N],
                            rsb[:, b * N:(b + 1) * N], op=AL.mult)
                    # v-bias for the whole chunk
                    nc.vector.tensor_scalar(
                        oT[:, qcol:qcol + F], oT[:, qcol:qcol + F],
                        qkvb[:, 12 + hp:13 + hp], None, op0=AL.add)

                # proj + residual
                for co in range(CH):
                    ps = psA()
                    for k in range(CH):
                        nc.tensor.matmul(
                            ps[:, 0:F],
                            wpj[:, k * C + co * 128: k * C + co * 128 + 128],
                            oT[:, k * F:(k + 1) * F],
                            start=(k == 0), stop=(k == CH - 1),
                        )
                    nc.vector.scalar_tensor_tensor(
                        xt[p][:, co * F:(co + 1) * F],
                        ps[:, 0:F], projb[:, co:co + 1],
                        xt[p][:, co * F:(co + 1) * F].bitcast(f32),
                        op0=AL.add, op1=AL.add)

        tap(1 + 2 * l, xt[0], F)

        # ---------------- pruning (after layer-SEL attention residual) --------
        if l == SEL:
            _prune(nc, tc, xt, unc, ident, ltt, iota, psB, psC, d)

        F = F0 if l < SEL else F1

        fc1b = load_bias(d["fc1b_d"], l, 24)
        fc2b = load_bias(d["fc2b_d"], l, 6)

        # ---------------- phase B: LN2 + MLP in 4 quarters ---------------------
        with tc.tile_pool(name="wB", bufs=1) as wB, tc.tile_pool(name="tB", bufs=1) as tB:
            xh2 = [layernorm(tB, xt[p], F, f"ln2_{p}") for p in range(PAIRS)]
            h1 = [tB.tile([128, CH * F], f16, tag=f"h1_{p}", name=f"h1_{p}") for p in range(PAIRS)]
            for q in range(4):
                w1 = wB.tile([128, CH * C], f16, tag="wfc1", bufs=1)
                nc.sync.dma_start(
                    w1[:].rearrange("p (k n) -> p k n", k=CH),
                    d["fc1w_d"][l][:, q * C:(q + 1) * C].rearrange("(k p) n -> p k n", p=128))
                w2 = wB.tile([128, CH * C], f16, tag="wfc2", bufs=1)
                nc.sync.dma_start(
                    w2[:].rearrange("p (k n) -> p k n", k=CH),
                    d["fc2w_d"][l][q * C:(q + 1) * C, :].rearrange("(k p) n -> p k n", p=128))
                for p in range(PAIRS):
                    for co in range(CH):
                        ps = psA()
                        for k in range(CH):
                            nc.tensor.matmul(
                                ps[:, 0:F],
                                w1[:, k * C + co * 128: k * C + co * 128 + 128],
                                xh2[p][:, k * F:(k + 1) * F],
                                start=(k == 0), stop=(k == CH - 1),
                            )
                        nc.scalar.activation(
                            h1[p][:, co * F:(co + 1) * F], ps[:, 0:F],
                            AF.Gelu, bias=fc1b[:, q * CH + co:q * CH + co + 1])
                    for co in range(CH):
                        ps = psA()
                        for k in range(CH):
                            nc.tensor.matmul(
                                ps[:, 0:F],
                                w2[:, k * C + co * 128: k * C + co * 128 + 128],
                                h1[p][:, k * F:(k + 1) * F],
                                start=(k == 0), stop=(k == CH - 1),
                            )
                        if q == 0:
                            nc.vector.scalar_tensor_tensor(
                                xt[p][:, co * F:(co + 1) * F],
                                ps[:, 0:F], fc2b[:, co:co + 1],
                                xt[p][:, co * F:(co + 1) * F].bitcast(f32),
                                op0=AL.add, op1=AL.add)
                        else:
                            nc.vector.tensor_tensor(
                                xt[p][:, co * F:(co + 1) * F],
                                ps[:, 0:F],
                                xt[p][:, co * F:(co + 1) * F].bitcast(f32),
                                op=AL.add)
        tap(2 + 2 * l, xt[0], F)

    # ------------------------------------------------------------ head
    with tc.tile_pool(name="whead", bufs=1) as wh, tc.tile_pool(name="thead", bufs=1) as th:
        clsT = th.tile([128, CH * B_CORE], f32r, tag="clsT")
        for p in range(PAIRS):
            for b in range(2):
                for k in range(CH):
                    nc.vector.tensor_copy(
                        clsT[:, k * B_CORE + 2 * p + b: k * B_CORE + 2 * p + b + 1],
                        xt[p][:, k * F1 + b * N1: k * F1 + b * N1 + 1])
        xhc = layernorm(th, clsT, B_CORE, "lnf")
        hw = wh.tile([128, CH * NCLS], f16, tag="hw")
        nc.sync.dma_start(hw[:].rearrange("p (k n) -> p k n", k=CH), d["headw_d"].rearrange("(k p) n -> p k n", p=128))
        hb = wh.tile([NCLS, 1], f32, tag="hb")
        nc.sync.dma_start(hb[:], d["headb_d"][:])
        ps = psC()
        for k in range(CH):
            nc.tensor.matmul(
                ps[0:NCLS, 0:B_CORE],
                hw[:, k * NCLS:(k + 1) * NCLS],
                xhc[:, k * B_CORE:(k + 1) * B_CORE],
                start=(k == 0), stop=(k == CH - 1),
            )
        lt = th.tile([NCLS, B_CORE], f32, tag="logits")
        nc.vector.tensor_scalar(lt[:], ps[0:NCLS, 0:B_CORE], hb[:, 0:1], None, op0=AL.add)
        nc.sync.dma_start(d["out_d"][:], lt[:])

    es.close()


def _prune(nc, tc, xt, unc, ident, ltt, iota, psB, psC, d):
    """Keep the K_KEEP lowest-uncertainty image tokens (drop the N_DROP
    highest), append mean of dropped; rewrite x in-place to [128, CH*F1]."""
    f32, f32r = dt.float32, dt.float32r
    jl = [128, 68]          # img-token chunk lengths (196 = 128 + 68)
    with tc.tile_pool(name="tprune", bufs=1) as tp:
        U = tp.tile([B_CORE, 196], f32, tag="U")
        for p in range(PAIRS):
            for b in range(2):
                # DVE writes must start at a 32-aligned partition; use DMA
                nc.sync.dma_start(
                    U[2 * p + b:2 * p + b + 1, :],
                    unc[p][:, b * N0 + 1:(b + 1) * N0])
        # drop mask: top-N_DROP largest per row (unc ~ 1, min_val 0 is safe;
        # mask threshold min(.,1) needs kept residuals >= 1?  values here are
        # sums of 12 reciprocals in (0,1): ~0.6..1.2 -- scale first to be safe.
        nc.vector.tensor_scalar(U[:], U[:], 100.0, None, op0=AL.mult)
        work = tp.tile([B_CORE, 196], f32, tag="work")
        mx = tp.tile([B_CORE, 8], f32, tag="mx")
        cur = U
        for k_on in range(0, N_DROP, 8):
            nfind = min(k_on + 8, N_DROP) - k_on
            nc.vector.max(out=mx[:], in_=cur[:])
            if nfind < 8:
                nc.vector.memset(mx[:, nfind:], 0.0)
            nc.vector.match_replace(out=work[:], in_to_replace=mx[:],
                                    in_values=cur[:], imm_value=0.0)
            cur = work
        nc.vector.tensor_sub(work[:], U[:], work[:])
        nc.vector.tensor_scalar_min(work[:], work[:], 1.0)   # drop mask {0,1}
        keep = tp.tile([B_CORE, 196], f32, tag="keep")
        nc.vector.tensor_scalar(keep[:], work[:], -1.0, 1.0, op0=AL.mult, op1=AL.add)
        if d.get("dbgp_d") is not None:
            nc.sync.dma_start(d["dbgp_d"][0][0:8, :], U[:])
            nc.sync.dma_start(d["dbgp_d"][1][0:8, :], keep[:])

        # keepT chunks via PE transpose
        keepT = [tp.tile([128, B_CORE], f32r, tag=f"keepT{i}", name=f"keepT{i}") for i in range(2)]
        for i in range(2):
            pt = psB()
            nc.tensor.transpose(pt[0:jl[i], 0:B_CORE],
                                keep[:, i * 128:i * 128 + jl[i]],
                                ident[0:B_CORE, 0:B_CORE])
            nc.vector.tensor_copy(keepT[i][0:jl[i], :], pt[0:jl[i], 0:B_CORE])
        # ranks = inclusive cumsum of keep via lower-triangular ones matmul
        prk = psC()
        for i in range(2):
            nc.tensor.matmul(
                prk[0:B_CORE, 0:196], keepT[i][0:jl[i], :],
                ltt[0:jl[i], i * 196:(i + 1) * 196],
                start=(i == 0), stop=(i == 1))
        ranks = tp.tile([B_CORE, 196], f32, tag="ranks")
        nc.vector.tensor_copy(ranks[:], prk[0:B_CORE, 0:196])
        if d.get("dbgp_d") is not None:
            nc.sync.dma_start(d["dbgp_d"][2][0:8, :], ranks[:])
        # target col t = keep*rank + (1-keep)*138 ; weight w = keep + (1-keep)/59
        tcol = tp.tile([B_CORE, 196], f32, tag="tcol")
        nc.vector.tensor_tensor(tcol[:], ranks[:], keep[:], op=AL.mult)
        nc.vector.scalar_tensor_tensor(tcol[:], keep[:], -float(N1 - 1), tcol[:],
                                       op0=AL.mult, op1=AL.add)
        nc.vector.tensor_scalar(tcol[:], tcol[:], float(N1 - 1), None, op0=AL.add)
        wcol = tp.tile([B_CORE, 196], f32, tag="wcol")
        nc.vector.tensor_scalar(wcol[:], keep[:], float((N_DROP - 1) / N_DROP),
                                1.0 / N_DROP, op0=AL.mult, op1=AL.add)
        tT = [tp.tile([128, B_CORE], f32, tag=f"tT{i}", name=f"tT{i}") for i in range(2)]
        wT = [tp.tile([128, B_CORE], f32, tag=f"wT{i}", name=f"wT{i}") for i in range(2)]
        for i in range(2):
            pt = psB()
            nc.tensor.transpose(pt[0:jl[i], 0:B_CORE],
                                tcol[:, i * 128:i * 128 + jl[i]],
                                ident[0:B_CORE, 0:B_CORE])
            nc.vector.tensor_copy(tT[i][0:jl[i], :], pt[0:jl[i], 0:B_CORE])
            pt2 = psB()
            nc.tensor.transpose(pt2[0:jl[i], 0:B_CORE],
                                wcol[:, i * 128:i * 128 + jl[i]],
                                ident[0:B_CORE, 0:B_CORE])
            nc.vector.tensor_copy(wT[i][0:jl[i], :], pt2[0:jl[i], 0:B_CORE])

        # per pair: transpose old x (img tokens only, cls-skipped so chunks
        # align with P), cls copies, then one-hot gather matmul, in place.
        for p in range(PAIRS):
            xa = xt[p]
            xtok = {}
            for b in range(2):
                for i in range(2):
                    tlen = jl[i]
                    xk = tp.tile([128, CH * 128], f32r, tag=f"xtok{b}{i}")
                    xtok[(b, i)] = xk
                    for k in range(CH):
                        pt = psB()
                        nc.tensor.transpose(
                            pt[0:tlen, 0:128],
                            xa[:, k * F0 + b * N0 + 1 + i * 128:
                               k * F0 + b * N0 + 1 + i * 128 + tlen].bitcast(f32),
                            ident[:])
                        nc.vector.tensor_copy(xk[0:tlen, k * 128:(k + 1) * 128],
                                              pt[0:tlen, 0:128])
            for b in range(2):
                for k in range(CH):
                    nc.vector.tensor_copy(
                        xa[:, k * F1 + b * N1: k * F1 + b * N1 + 1],
                        xa[:, k * F0 + b * N0: k * F0 + b * N0 + 1])
            for b in range(2):
                img = 2 * p + b
                P = [tp.tile([128, N1 - 1], f32r, tag=f"P{i}", name=f"P{i}") for i in range(2)]
                for i in range(2):
                    nc.vector.tensor_scalar(
                        P[i][0:jl[i], :], iota[0:jl[i], :],
                        tT[i][0:jl[i], img:img + 1], wT[i][0:jl[i], img:img + 1],
                        op0=AL.is_equal, op1=AL.mult)
                for k in range(CH):
                    pg = psC()
                    for i in range(2):
                        nc.tensor.matmul(
                            pg[0:128, 0:N1 - 1],
                            xtok[(b, i)][0:jl[i], k * 128:(k + 1) * 128],
                            P[i][0:jl[i], :],
                            start=(i == 0), stop=(i == 1))
                    nc.vector.tensor_copy(
                        xa[:, k * F1 + b * N1 + 1: k * F1 + b * N1 + N1],
                        pg[0:128, 0:N1 - 1])


# ------------------------------------------------------------------- host
def _host_pack(inputs):
    """Fold LN affines into weights, pre-transpose, pre-extract patches."""
    f = np.float32
    h = np.float16
    inp = {k: np.asarray(v, f) for k, v in inputs.items()}
    out = {}

    imgs = inp['inputs']
    B = imgs.shape[0]
    x = imgs.reshape(B, 3, 14, 16, 14, 16).transpose(0, 2, 4, 1, 3, 5).reshape(B, 196, 768)
    out['patchesT_full'] = np.ascontiguousarray(x.transpose(2, 0, 1).reshape(768, B * 196))

    posC = inp['pos_embed'][0].copy()
    posC[0] += inp['cls_token'][0, 0]
    posC[1:] += inp['patch_b'][None, :]
    out['posCT'] = np.ascontiguousarray(posC.T)

    out['patch_wT'] = np.ascontiguousarray(inp['patch_w'].reshape(C, -1).T)

    qkv_wT = np.empty((DEPTH, C, 3 * C), h)
    qkv_bL = np.empty((DEPTH, 128, 18), f)
    proj_wT = np.empty((DEPTH, C, C), h)
    proj_bL = np.empty((DEPTH, 128, 6), f)
    fc1_wT = np.empty((DEPTH, C, 4 * C), h)
    fc1_bL = np.empty((DEPTH, 128, 24), f)
    fc2_wT = np.empty((DEPTH, 4 * C, C), h)
    fc2_bL = np.empty((DEPTH, 128, 6), f)
    for l in range(DEPTH):
        w1 = inp['qkv_w'][l] * inp['ln1_g'][l][None, :]
        b1 = inp['qkv_b'][l] + inp['qkv_w'][l] @ inp['ln1_b'][l]
        qkv_wT[l] = w1.T.astype(h)
        qkv_bL[l] = b1.reshape(18, 128).T
        proj_wT[l] = inp['proj_w'][l].T.astype(h)
        proj_bL[l] = inp['proj_b'][l].reshape(6, 128).T
        wf1 = inp['fc1_w'][l] * inp['ln2_g'][l][None, :]
        bf1 = inp['fc1_b'][l] + inp['fc1_w'][l] @ inp['ln2_b'][l]
        fc1_wT[l] = wf1.T.astype(h)
        fc1_bL[l] = bf1.reshape(24, 128).T
        fc2_wT[l] = inp['fc2_w'][l].T.astype(h)
        fc2_bL[l] = inp['fc2_b'][l].reshape(6, 128).T
    out.update(qkv_wT=qkv_wT, qkv_bL=qkv_bL, proj_wT=proj_wT, proj_bL=proj_bL,
               fc1_wT=fc1_wT, fc1_bL=fc1_bL, fc2_wT=fc2_wT, fc2_bL=fc2_bL)

    hw = inp['head_w'] * inp['norm_g'][None, :]
    hb = inp['head_b'] + inp['head_w'] @ inp['norm_b']
    out['headT'] = np.ascontiguousarray(hw.T.astype(h))
    out['head_bL'] = np.ascontiguousarray(hb.reshape(NCLS, 1))

    out['ident'] = np.eye(128, dtype=f)
    out['ones'] = np.ones((128, 128), h)
    out['invC'] = np.full((128, 128), 1.0 / C, f)
    out['iota'] = np.tile(np.arange(1, N1, dtype=f), (128, 1))
    out['LT'] = (np.arange(196)[:, None] <= np.arange(196)[None, :]).astype(f)
    return out


_BUILT = None


def kernel(**inputs):
    global _BUILT
    host = _host_pack(inputs)
    if _BUILT is None:
        nc = build_nc()
        split_excess_waits(nc)
        _BUILT = nc
    nc = _BUILT

    shared_keys = ['posCT', 'patch_wT', 'qkv_wT', 'qkv_bL', 'proj_wT', 'proj_bL',
                   'fc1_wT', 'fc1_bL', 'fc2_wT', 'fc2_bL', 'headT', 'head_bL',
                   'ident', 'ones', 'invC', 'iota', 'LT']
    in_maps = []
    for c in range(NCORES):
        m = {k: host[k] for k in shared_keys}
        m['patchesT'] = np.ascontiguousarray(
            host['patchesT_full'][:, c * B_CORE * 196:(c + 1) * B_CORE * 196])
        in_maps.append(m)

    trace = bool(os.environ.get("BASS_VIT_TRACE"))
    res = run_bass_kernel_spmd(nc, in_maps, core_ids=list(range(NCORES)), trace=trace)
    if trace:
        print(f"HW exec time: {res.exec_time_ns} ns (mean {res.mean_exec_time_ns})")
        kernel.last_exec_time_ns = res.exec_time_ns

    out = np.concatenate([res.results[c]["logitsT"].T for c in range(NCORES)],
                         axis=0).astype(np.float32)
    if os.environ.get("BASS_VIT_DEBUG_LAYER", ""):
        kernel.last_dbg = [res.results[c].get("dbg") for c in range(NCORES)]
        kernel.last_dbgp = [res.results[c].get("dbgp") for c in range(NCORES)]
    return out


# revision 23
# speedup vs baseline: 1.0064x; 1.0064x over previous
"""CertViT (ViT-Base + layer-3 token pruning) forward pass on 8 Trainium2 cores.

Data parallel: 8 images per core, processed as 4 image-pairs so dense matmul
free dims (394 / 278) stay >= 256. Activations live in channel-partition
layout x^T [768 -> 6x128 chunks, tokens]; the residual stream x stays fp32,
everything fed to the PE (post-LN activations, q/k/v, exp weights, weights)
is fp16 so small-free-dim attention matmuls run at 1 cycle/row and DVE ops
get the 2x/4x modes. Attention processes heads in even/odd pairs: QK is
row-tiled (contraction 64: even head rows 0:63, odd 64:127), the softmax
denominator and AV are col-tiled (output partitions 0:63 / 64:127), so head
pairs run concurrently in the PE array and odd heads no longer need a
partition-shift DMA. Both images of a pair share one QK PSUM bank
([keys, img0 queries | img1 queries]), halving exp instruction count and AV
streamed columns. All reciprocals (softmax denom, LN rsqrt, uncertainty) are
computed as Exp(-k*Ln(x)) on ScalarE -- one activation-table set shared with
the attention Exp, nothing iterative on DVE. LayerNorm affine params are
folded into the following matmul weights on the host. Top-k pruning uses
max8/match_replace for the drop mask, a triangular-matmul cumsum for ranks,
and a one-hot permutation matmul for the gather.
"""

import os
import sys

import numpy as np

for _p in ('/opt/trn_rl_repo', '/root/.axon_site/_ro/trn_rl_repo'):
    if os.path.isdir(_p) and _p not in sys.path:
        sys.path.append(_p)

import concourse.bass as bass
import concourse.mybir as mybir
from concourse.tile import TileContext
from concourse.bass_utils import run_bass_kernel_spmd
from concourse.alu_op_type import AluOpType as AL

dt = mybir.dt
AF = mybir.ActivationFunctionType

# ---------------------------------------------------------------- config
NCORES = 8
B_CORE = 8            # images per core
PAIRS = B_CORE // 2
C = 768
CH = C // 128          # 6 channel chunks
HD = 12                # heads
HP = HD // 2           # head pairs
D = 64                 # head dim
SCALE = D ** -0.5
DEPTH = 12
SEL = 3                # pruning layer
N0 = 197               # tokens before pruning
K_KEEP = 137           # int(197*0.7)
N_DROP = N0 - 1 - K_KEEP   # 59
N1 = K_KEEP + 2        # 139 tokens after pruning
F0 = 2 * N0            # pair free dim, layers 0..3
F1 = 2 * N1            # pair free dim, layers 4..11
EPS = 1e-6
NCLS = 100

# ------------------------------------------------------------- waitfix
# This walrus build accepts at most ONE sem wait per instruction; Tile can
# attach several. Move excess waits onto InstNoOp carriers inserted before.
_wf_counter = [0]


def _wf_carrier(engine, waits):
    _wf_counter[0] += 1
    d = mybir.InstNoOp(name=f"waitfix-{_wf_counter[0]}", ins=[], outs=[])
    d.engine = engine
    d.sync_info = mybir.SyncInfo(on_wait=list(waits), on_update=[])
    return d


def split_excess_waits(nc, max_waits=1):
    nfix = 0
    for f in nc.m.functions:
        for bb in f.blocks:
            insts = list(bb.instructions)
            out = []
            changed = False
            for inst in insts:
                si = inst.sync_info
                waits = list(si.on_wait) if si and si.on_wait else []
                if len(waits) > max_waits:
                    keep, rest = waits[:max_waits], waits[max_waits:]
                    while rest:
                        chunk, rest = rest[:max_waits], rest[max_waits:]
                        out.append(_wf_carrier(inst.engine, chunk))
                    si.on_wait = keep
                    changed = True
                    nfix += 1
                out.append(inst)
            if changed:
                bb.instructions = out
    return nfix


# ----------------------------------------------------------- device kernel
def build_nc():
    nc = bass.Bass()
    f32, f32r, f16 = dt.float32, dt.float32r, dt.float16

    d = {}
    d["patches_d"] = nc.declare_dram_parameter("patchesT", [C, B_CORE * 196], f32r, isOutput=False)
    d["posc_d"] = nc.declare_dram_parameter("posCT", [C, N0], f32, isOutput=False)
    d["pw_d"] = nc.declare_dram_parameter("patch_wT", [C, C], f32r, isOutput=False)
    d["qkvw_d"] = nc.declare_dram_parameter("qkv_wT", [DEPTH, C, 3 * C], f16, isOutput=False)
    d["qkvb_d"] = nc.declare_dram_parameter("qkv_bL", [DEPTH, 128, 18], f32, isOutput=False)
    d["projw_d"] = nc.declare_dram_parameter("proj_wT", [DEPTH, C, C], f16, isOutput=False)
    d["projb_d"] = nc.declare_dram_parameter("proj_bL", [DEPTH, 128, 6], f32, isOutput=False)
    d["fc1w_d"] = nc.declare_dram_parameter("fc1_wT", [DEPTH, C, 4 * C], f16, isOutput=False)
    d["fc1b_d"] = nc.declare_dram_parameter("fc1_bL", [DEPTH, 128, 24], f32, isOutput=False)
    d["fc2w_d"] = nc.declare_dram_parameter("fc2_wT", [DEPTH, 4 * C, C], f16, isOutput=False)
    d["fc2b_d"] = nc.declare_dram_parameter("fc2_bL", [DEPTH, 128, 6], f32, isOutput=False)
    d["headw_d"] = nc.declare_dram_parameter("headT", [C, NCLS], f16, isOutput=False)
    d["headb_d"] = nc.declare_dram_parameter("head_bL", [NCLS, 1], f32, isOutput=False)
    d["ident_d"] = nc.declare_dram_parameter("ident", [128, 128], f32, isOutput=False)
    d["ones_d"] = nc.declare_dram_parameter("ones", [128, 128], f16, isOutput=False)
    d["invc_d"] = nc.declare_dram_parameter("invC", [128, 128], f32r, isOutput=False)
    d["invc16_d"] = nc.declare_dram_parameter("invC16", [128, 128], f16, isOutput=False)
    d["iota_d"] = nc.declare_dram_parameter("iota", [128, N1 - 1], f32, isOutput=False)
    d["lt_d"] = nc.declare_dram_parameter("LT", [196, 196], f32r, isOutput=False)
    d["out_d"] = nc.declare_dram_parameter("logitsT", [NCLS, B_CORE], f32, isOutput=True)

    d["dbg_layer"] = os.environ.get("BASS_VIT_DEBUG_LAYER", "")
    if d["dbg_layer"]:
        d["dbg_d"] = nc.declare_dram_parameter("dbg", [1 + 2 * DEPTH, 128, CH * F0], f32, isOutput=True)
        d["dbgp_d"] = nc.declare_dram_parameter("dbgp", [4, 8, 196], f32, isOutput=True)
    else:
        d["dbg_d"] = None
        d["dbgp_d"] = None

    with TileContext(nc) as tc:
        _build_body(nc, tc, d)
    return nc


def _build_body(nc, tc, d):
    f32, f32r, f16 = dt.float32, dt.float32r, dt.float16
    from contextlib import ExitStack
    es = ExitStack()

    cpool = es.enter_context(tc.tile_pool(name="consts", bufs=1))
    xpool = es.enter_context(tc.tile_pool(name="x", bufs=1))
    ppool = es.enter_context(tc.tile_pool(name="psum", bufs=1, space="PSUM"))
    prpool = es.enter_context(tc.tile_pool(name="prune", bufs=1))
    bpool = es.enter_context(tc.tile_pool(name="bias", bufs=2))

    # constants
    ident = cpool.tile([128, 128], f32, tag="ident")
    ones = cpool.tile([128, 128], f16, tag="ones")
    invc = cpool.tile([128, 128], f32r, tag="invc")
    invc16 = cpool.tile([128, 128], f16, tag="invc16")
    iota = cpool.tile([128, N1 - 1], f32, tag="iota")
    ltt = cpool.tile([128, 2 * 196], f32r, tag="ltt")
    posct = cpool.tile([128, CH * N0], f32, tag="posct")
    eps_t = cpool.tile([128, 1], f32, tag="eps_t")
    nc.vector.memset(eps_t[:], EPS)
    n0_t = cpool.tile([128, 1], f32, tag="n0_t")
    nc.vector.memset(n0_t[:], float(N0))
    nc.sync.dma_start(ident[:], d["ident_d"][:])
    nc.sync.dma_start(ones[:], d["ones_d"][:])
    nc.sync.dma_start(invc[:], d["invc_d"][:])
    nc.sync.dma_start(invc16[:], d["invc16_d"][:])
    nc.sync.dma_start(iota[:], d["iota_d"][:])
    nc.sync.dma_start(ltt[:, 0:196], d["lt_d"][0:128, :])
    nc.sync.dma_start(ltt[0:68, 196:392], d["lt_d"][128:196, :])
    nc.sync.dma_start(posct[:].rearrange("p (k n) -> p k n", k=CH), d["posc_d"].rearrange("(k p) n -> p k n", p=128))

    # PSUM slots: tag 'a' x3 (main accumulations + QK), 'b' x2 (denominator),
    # 'c' x3 (AV / LN meansq) -> 8 banks
    def psA():
        return ppool.tile([128, F0], f32, tag="a", bufs=3, name="psA")

    def psB():
        return ppool.tile([128, F0], f32, tag="b", bufs=2, name="psB")

    def psC():
        return ppool.tile([128, F0], f32, tag="c", bufs=3, name="psC")

    # persistent per-pair residual stream x^T, chunk-major [128, CH*F]
    xt = [xpool.tile([128, CH * F0], f32r, tag=f"x{p}", name=f"x{p}") for p in range(PAIRS)]
    # per-pair uncertainty rows (filled at layer SEL)
    unc = [prpool.tile([1, F0], f32, tag=f"unc{p}", name=f"unc{p}") for p in range(PAIRS)]

    # ------------------------------------------------------------ patch embed
    with tc.tile_pool(name="wpatch", bufs=1) as wp, tc.tile_pool(name="tpatch", bufs=2) as tp:
        pwt = wp.tile([128, CH * C], f32r, tag="pw")
        nc.sync.dma_start(pwt[:].rearrange("p (k n) -> p k n", k=CH), d["pw_d"].rearrange("(k p) n -> p k n", p=128))
        for p in range(PAIRS):
            prt = tp.tile([128, CH * 392], f32r, tag="patches")
            nc.sync.dma_start(
                prt[:].rearrange("p (k n) -> p k n", k=CH),
                d["patches_d"][:, p * 392:(p + 1) * 392].rearrange("(k p) n -> p k n", p=128),
            )
            for co in range(CH):
                ps = psA()
                for k in range(CH):
                    nc.tensor.matmul(
                        ps[:, 0:392],
                        pwt[:, k * C + co * 128: k * C + co * 128 + 128],
                        prt[:, k * 392:(k + 1) * 392],
                        start=(k == 0), stop=(k == CH - 1),
                    )
                for b in range(2):
                    nc.vector.tensor_tensor(
                        xt[p][:, co * F0 + b * N0 + 1: co * F0 + b * N0 + N0],
                        ps[:, b * 196:(b + 1) * 196],
                        posct[:, co * N0 + 1: co * N0 + N0],
                        op=AL.add,
                    )
                    nc.vector.tensor_copy(
                        xt[p][:, co * F0 + b * N0: co * F0 + b * N0 + 1],
                        posct[:, co * N0: co * N0 + 1],
                    )

    def tap(slot, xtile, F):
        if d["dbg_d"] is not None:
            nc.sync.dma_start(d["dbg_d"][slot][:, 0:CH * F], xtile[:, 0:CH * F].bitcast(f32))

    tap(0, xt[0], F0)

    # ------------------------------------------------------------ helpers
    def layernorm(pool, x, F, xh_tag, xh_bufs=1):
        """Standardize x (chunk-major [128, CH*F]) per token -> fp16 tile."""
        xh = pool.tile([128, CH * F], f16, tag=xh_tag, bufs=xh_bufs, name=xh_tag)
        sq = pool.tile([128, CH * F], f16, tag="ln_sq", bufs=2)
        for k in range(CH):
            nc.scalar.activation(
                sq[:, k * F:(k + 1) * F],
                x[:, k * F:(k + 1) * F].bitcast(f32), AF.Square)
        pm = psB()
        ps2 = psC()
        for k in range(CH):
            nc.tensor.matmul(pm[:, 0:F], invc[:], x[:, k * F:(k + 1) * F],
                             start=(k == 0), stop=(k == CH - 1))
        for k in range(CH):
            nc.tensor.matmul(ps2[:, 0:F], invc16[:], sq[:, k * F:(k + 1) * F],
                             start=(k == 0), stop=(k == CH - 1))
        var = pool.tile([128, F], f32, tag="ln_var", bufs=2)
        rstd = pool.tile([128, F], f32, tag="ln_rstd", bufs=2)
        mean = pool.tile([128, F], f32, tag="ln_mean", bufs=2)
        nc.vector.tensor_copy(mean[:], pm[:, 0:F])
        nc.vector.tensor_tensor(var[:], mean[:], mean[:], op=AL.mult)
        nc.vector.tensor_tensor(var[:], ps2[:, 0:F], var[:], op=AL.subtract)
        # rstd = exp(-0.5*ln(var+eps)) = 1/sqrt(var+eps); Ln+Exp share one
        # activation-table set with the attention Exp.
        nc.scalar.activation(rstd[:], var[:], AF.Ln, bias=eps_t[:, 0:1])
        nc.scalar.activation(rstd[:], rstd[:], AF.Exp, scale=-0.5)
        for k in range(CH):
            nc.vector.tensor_tensor(
                var[:], x[:, k * F:(k + 1) * F].bitcast(f32), mean[:], op=AL.subtract)
            nc.vector.tensor_tensor(
                xh[:, k * F:(k + 1) * F], var[:], rstd[:], op=AL.mult)
        return xh

    def load_bias(dram_t, l, cols):
        bt = bpool.tile([128, cols], f32, tag=dram_t.name)
        nc.sync.dma_start(bt[:], dram_t[l])
        return bt

    # ------------------------------------------------------------ layers
    for l in range(DEPTH):
        F = F0 if l <= SEL else F1
        N = N0 if l <= SEL else N1
        mlens = [128, N - 128]

        qkvb = load_bias(d["qkvb_d"], l, 18)
        projb = load_bias(d["projb_d"], l, 6)

        # ---------------- phase A: LN1 + QKV + attention + proj ----------------
        with tc.tile_pool(name="wA", bufs=1) as wA, tc.tile_pool(name="tA", bufs=1) as tA:
            wq = wA.tile([128, CH * 3 * C], f16, tag="wqkv")
            nc.sync.dma_start(wq[:].rearrange("p (k n) -> p k n", k=CH), d["qkvw_d"][l].rearrange("(k p) n -> p k n", p=128))
            wpj = wA.tile([128, CH * C], f16, tag="wproj")
            nc.sync.dma_start(wpj[:].rearrange("p (k n) -> p k n", k=CH), d["projw_d"][l].rearrange("(k p) n -> p k n", p=128))

            xhs = [layernorm(tA, xt[p], F, "ln1", xh_bufs=4) for p in range(PAIRS)]
            for p in range(PAIRS):
                xh = xhs[p]
                qT = tA.tile([128, CH * F], f16, tag="qT", bufs=2, name="qT")
                kT = tA.tile([128, CH * F], f16, tag="kT", bufs=2, name="kT")
                for o in range(12):
                    ps = psA()
                    for k in range(CH):
                        nc.tensor.matmul(
                            ps[:, 0:F],
                            wq[:, k * 3 * C + o * 128: k * 3 * C + o * 128 + 128],
                            xh[:, k * F:(k + 1) * F],
                            start=(k == 0), stop=(k == CH - 1),
                        )
                    oc = o % CH
                    # SCALE is folded into the q weights/bias on the host
                    dst = qT if o < CH else kT
                    nc.scalar.activation(
                        dst[:, oc * F:(oc + 1) * F], ps[:, 0:F],
                        AF.Identity, bias=qkvb[:, o:o + 1])

                # v in token-partition layout, per image: 2 t-chunks
                vto = [[None, None], [None, None]]
                for b in range(2):
                    for tchunk in range(2):
                        tlen = mlens[tchunk]
                        toff = b * N + tchunk * 128
                        vt = tA.tile([128, C], f16, tag=f"v{b}{tchunk}", bufs=2)
                        vto[b][tchunk] = vt
                        for half in range(2):
                            ps = psA()
                            for k in range(CH):
                                nc.tensor.matmul(
                                    ps[0:tlen, 0:384],
                                    xh[:, k * F + toff: k * F + toff + tlen],
                                    wq[:, k * 3 * C + 2 * C + half * 384:
                                       k * 3 * C + 2 * C + half * 384 + 384],
                                    start=(k == 0), stop=(k == CH - 1),
                                )
                            nc.scalar.activation(
                                vt[0:tlen, half * 384:(half + 1) * 384],
                                ps[0:tlen, 0:384], AF.Copy)

                # attention by head pair hp: even head e=0 on rows/out-cols
                # 0:64, odd e=1 on 64:128 (row-tiled QK, col-tiled denom/AV).
                # Pass 1 (all hp): QK + exp; pass 2 (all hp): denom/AV/norm --
                # keeps the PE stream free of exp-latency head-of-line stalls.
                oT = tA.tile([128, CH * F], f16, tag="oT", bufs=2, name="oT")
                ets = {}
                pevs = {}

                def qk_pass(hp):
                    qcol = hp * F
                    et = [[tA.tile([128, F], f16, tag=f"et{e}{t}", bufs=6,
                                   name=f"et{e}{t}") for t in range(2)]
                          for e in range(2)]
                    ets[hp] = et
                    if l == SEL:
                        pevs[hp] = [psB(), psA()]
                    for tchunk in range(2):
                        tlen = mlens[tchunk]
                        toff = tchunk * 128
                        psQK = [psA(), psA()]
                        for b in range(2):
                            for e in range(2):
                                nc.tensor.matmul(
                                    psQK[e][0:tlen, b * N:(b + 1) * N],
                                    kT[e * 64:e * 64 + 64,
                                       qcol + b * N + toff: qcol + b * N + toff + tlen],
                                    qT[e * 64:e * 64 + 64, qcol + b * N: qcol + (b + 1) * N],
                                    start=True, stop=True,
                                )
                        for e in range(2):
                            nc.scalar.activation(
                                et[e][tchunk][0:tlen, 0:F],
                                psQK[e][0:tlen, 0:F], AF.Exp)
                        if l == SEL:
                            for e in range(2):
                                rt = tA.tile([128, F], f16, tag=f"relu{e}", bufs=1)
                                nc.vector.tensor_scalar(
                                    rt[0:tlen, 0:F], psQK[e][0:tlen, 0:F],
                                    0.0, None, op0=AL.max)
                                nc.tensor.matmul(
                                    pevs[hp][e][0:1, 0:F], ones[0:tlen, 0:1],
                                    rt[0:tlen, 0:F],
                                    start=(tchunk == 0), stop=(tchunk == 1),
                                )

                def av_pass(hp):
                    qcol = hp * F
                    et = ets.pop(hp)
                    pden = psB()
                    # one AV bank per image; parities col-tiled on disjoint
                    # partitions (0:64 / 64:128), so their accumulation groups
                    # interleave safely within a bank.
                    pav = [psC(), psC()]
                    for tchunk in range(2):
                        tlen = mlens[tchunk]
                        for e in range(2):
                            nc.tensor.matmul(
                                pden[e * 64:e * 64 + 64, 0:F],
                                ones[0:tlen, 0:64],
                                et[e][tchunk][0:tlen, 0:F],
                                start=(tchunk == 0), stop=(tchunk == 1),
                            )
                        for b in range(2):
                            for e in range(2):
                                nc.tensor.matmul(
                                    pav[b][e * 64:e * 64 + 64, 0:N],
                                    vto[b][tchunk][0:tlen,
                                                   (2 * hp + e) * 64:(2 * hp + e) * 64 + 64],
                                    et[e][tchunk][0:tlen, b * N:(b + 1) * N],
                                    start=(tchunk == 0), stop=(tchunk == 1),
                                )
                    if l == SEL:
                        # unc += 1/(evidence_sum + N) per head
                        for e in range(2):
                            ev1 = tA.tile([1, F], f32, tag="ev1", bufs=2)
                            nc.scalar.activation(
                                ev1[:], pevs[hp][e][0:1, 0:F], AF.Ln,
                                bias=n0_t[0:1, 0:1])
                            nc.scalar.activation(ev1[:], ev1[:], AF.Exp, scale=-1.0)
                            if hp == 0 and e == 0:
                                nc.vector.tensor_copy(unc[p][:], ev1[:])
                            else:
                                nc.vector.tensor_tensor(
                                    unc[p][:], ev1[:], unc[p][:], op=AL.add)
                        pevs.pop(hp)
                    # rsb = 1/denominator via exp(-ln), both parities at once
                    rsb = tA.tile([128, F], f32, tag="rsb", bufs=2)
                    nc.scalar.activation(rsb[:], pden[0:128, 0:F], AF.Ln)
                    nc.scalar.activation(rsb[:], rsb[:], AF.Exp, scale=-1.0)
                    for b in range(2):
                        nc.vector.tensor_tensor(
                            oT[:, qcol + b * N:qcol + (b + 1) * N],
                            pav[b][0:128, 0:N],
                            rsb[:, b * N:(b + 1) * N], op=AL.mult)
                    # v-bias for the whole chunk
                    nc.vector.tensor_scalar(
                        oT[:, qcol:qcol + F], oT[:, qcol:qcol + F],
                        qkvb[:, 12 + hp:13 + hp], None, op0=AL.add)

                if l == SEL:
                    # pev PSUM lifetimes don't allow the two-pass split here
                    for hp in range(HP):
                        qk_pass(hp)
                        av_pass(hp)
                else:
                    for hp in range(HP):
                        qk_pass(hp)
                    for hp in range(HP):
                        av_pass(hp)

                # proj + residual
                for co in range(CH):
                    ps = psA()
                    for k in range(CH):
                        nc.tensor.matmul(
                            ps[:, 0:F],
                            wpj[:, k * C + co * 128: k * C + co * 128 + 128],
                            oT[:, k * F:(k + 1) * F],
                            start=(k == 0), stop=(k == CH - 1),
                        )
                    nc.vector.scalar_tensor_tensor(
                        xt[p][:, co * F:(co + 1) * F],
                        ps[:, 0:F], projb[:, co:co + 1],
                        xt[p][:, co * F:(co + 1) * F].bitcast(f32),
                        op0=AL.add, op1=AL.add)

        tap(1 + 2 * l, xt[0], F)

        # ---------------- pruning (after layer-SEL attention residual) --------
        if l == SEL:
            _prune(nc, tc, xt, unc, ident, ltt, iota, psB, psC, d)

        F = F0 if l < SEL else F1

        fc1b = load_bias(d["fc1b_d"], l, 24)
        fc2b = load_bias(d["fc2b_d"], l, 6)

        # ---------------- phase B: LN2 + MLP in 4 quarters ---------------------
        with tc.tile_pool(name="wB", bufs=1) as wB, tc.tile_pool(name="tB", bufs=1) as tB:
            xh2 = [layernorm(tB, xt[p], F, f"ln2_{p}") for p in range(PAIRS)]
            h1 = [tB.tile([128, CH * F], f16, tag=f"h1_{p}", name=f"h1_{p}") for p in range(PAIRS)]
            for q in range(4):
                w1 = wB.tile([128, CH * C], f16, tag="wfc1", bufs=2)
                nc.sync.dma_start(
                    w1[:].rearrange("p (k n) -> p k n", k=CH),
                    d["fc1w_d"][l][:, q * C:(q + 1) * C].rearrange("(k p) n -> p k n", p=128))
                w2 = wB.tile([128, CH * C], f16, tag="wfc2", bufs=2)
                nc.sync.dma_start(
                    w2[:].rearrange("p (k n) -> p k n", k=CH),
                    d["fc2w_d"][l][q * C:(q + 1) * C, :].rearrange("(k p) n -> p k n", p=128))
                for p in range(PAIRS):
                    for co in range(CH):
                        ps = psA()
                        for k in range(CH):
                            nc.tensor.matmul(
                                ps[:, 0:F],
                                w1[:, k * C + co * 128: k * C + co * 128 + 128],
                                xh2[p][:, k * F:(k + 1) * F],
                                start=(k == 0), stop=(k == CH - 1),
                            )
                        nc.scalar.activation(
                            h1[p][:, co * F:(co + 1) * F], ps[:, 0:F],
                            AF.Gelu, bias=fc1b[:, q * CH + co:q * CH + co + 1])
                    for co in range(CH):
                        ps = psA()
                        for k in range(CH):
                            nc.tensor.matmul(
                                ps[:, 0:F],
                                w2[:, k * C + co * 128: k * C + co * 128 + 128],
                                h1[p][:, k * F:(k + 1) * F],
                                start=(k == 0), stop=(k == CH - 1),
                            )
                        if q == 0:
                            nc.vector.scalar_tensor_tensor(
                                xt[p][:, co * F:(co + 1) * F],
                                ps[:, 0:F], fc2b[:, co:co + 1],
                                xt[p][:, co * F:(co + 1) * F].bitcast(f32),
                                op0=AL.add, op1=AL.add)
                        else:
                            nc.vector.tensor_tensor(
                                xt[p][:, co * F:(co + 1) * F],
                                ps[:, 0:F],
                                xt[p][:, co * F:(co + 1) * F].bitcast(f32),
                                op=AL.add)
        tap(2 + 2 * l, xt[0], F)

    # ------------------------------------------------------------ head
    with tc.tile_pool(name="whead", bufs=1) as wh, tc.tile_pool(name="thead", bufs=1) as th:
        clsT = th.tile([128, CH * B_CORE], f32r, tag="clsT")
        for p in range(PAIRS):
            for b in range(2):
                for k in range(CH):
                    nc.vector.tensor_copy(
                        clsT[:, k * B_CORE + 2 * p + b: k * B_CORE + 2 * p + b + 1],
                        xt[p][:, k * F1 + b * N1: k * F1 + b * N1 + 1])
        xhc = layernorm(th, clsT, B_CORE, "lnf")
        hw = wh.tile([128, CH * NCLS], f16, tag="hw")
        nc.sync.dma_start(hw[:].rearrange("p (k n) -> p k n", k=CH), d["headw_d"].rearrange("(k p) n -> p k n", p=128))
        hb = wh.tile([NCLS, 1], f32, tag="hb")
        nc.sync.dma_start(hb[:], d["headb_d"][:])
        ps = psC()
        for k in range(CH):
            nc.tensor.matmul(
                ps[0:NCLS, 0:B_CORE],
                hw[:, k * NCLS:(k + 1) * NCLS],
                xhc[:, k * B_CORE:(k + 1) * B_CORE],
                start=(k == 0), stop=(k == CH - 1),
            )
        lt = th.tile([NCLS, B_CORE], f32, tag="logits")
        nc.vector.tensor_scalar(lt[:], ps[0:NCLS, 0:B_CORE], hb[:, 0:1], None, op0=AL.add)
        nc.sync.dma_start(d["out_d"][:], lt[:])

    es.close()


def _prune(nc, tc, xt, unc, ident, ltt, iota, psB, psC, d):
    """Keep the K_KEEP lowest-uncertainty image tokens (drop the N_DROP
    highest), append mean of dropped; rewrite x in-place to [128, CH*F1]."""
    f32, f32r = dt.float32, dt.float32r
    jl = [128, 68]          # img-token chunk lengths (196 = 128 + 68)
    with tc.tile_pool(name="tprune", bufs=1) as tp:
        U = tp.tile([B_CORE, 196], f32, tag="U")
        for p in range(PAIRS):
            for b in range(2):
                # DVE writes must start at a 32-aligned partition; use DMA
                nc.sync.dma_start(
                    U[2 * p + b:2 * p + b + 1, :],
                    unc[p][:, b * N0 + 1:(b + 1) * N0])
        # drop mask: top-N_DROP largest per row (unc ~ 1, min_val 0 is safe;
        # mask threshold min(.,1) needs kept residuals >= 1?  values here are
        # sums of 12 reciprocals in (0,1): ~0.6..1.2 -- scale first to be safe.
        nc.vector.tensor_scalar(U[:], U[:], 100.0, None, op0=AL.mult)
        work = tp.tile([B_CORE, 196], f32, tag="work")
        mx = tp.tile([B_CORE, 8], f32, tag="mx")
        cur = U
        for k_on in range(0, N_DROP, 8):
            nfind = min(k_on + 8, N_DROP) - k_on
            nc.vector.max(out=mx[:], in_=cur[:])
            if nfind < 8:
                nc.vector.memset(mx[:, nfind:], 0.0)
            nc.vector.match_replace(out=work[:], in_to_replace=mx[:],
                                    in_values=cur[:], imm_value=0.0)
            cur = work
        nc.vector.tensor_sub(work[:], U[:], work[:])
        nc.vector.tensor_scalar_min(work[:], work[:], 1.0)   # drop mask {0,1}
        keep = tp.tile([B_CORE, 196], f32, tag="keep")
        nc.vector.tensor_scalar(keep[:], work[:], -1.0, 1.0, op0=AL.mult, op1=AL.add)
        if d.get("dbgp_d") is not None:
            nc.sync.dma_start(d["dbgp_d"][0][0:8, :], U[:])
            nc.sync.dma_start(d["dbgp_d"][1][0:8, :], keep[:])

        # keepT chunks via PE transpose
        keepT = [tp.tile([128, B_CORE], f32r, tag=f"keepT{i}", name=f"keepT{i}") for i in range(2)]
        for i in range(2):
            pt = psB()
            nc.tensor.transpose(pt[0:jl[i], 0:B_CORE],
                                keep[:, i * 128:i * 128 + jl[i]],
                                ident[0:B_CORE, 0:B_CORE])
            nc.vector.tensor_copy(keepT[i][0:jl[i], :], pt[0:jl[i], 0:B_CORE])
        # ranks = inclusive cumsum of keep via lower-triangular ones matmul
        prk = psC()
        for i in range(2):
            nc.tensor.matmul(
                prk[0:B_CORE, 0:196], keepT[i][0:jl[i], :],
                ltt[0:jl[i], i * 196:(i + 1) * 196],
                start=(i == 0), stop=(i == 1))
        ranks = tp.tile([B_CORE, 196], f32, tag="ranks")
        nc.vector.tensor_copy(ranks[:], prk[0:B_CORE, 0:196])
        if d.get("dbgp_d") is not None:
            nc.sync.dma_start(d["dbgp_d"][2][0:8, :], ranks[:])
        # target col t = keep*rank + (1-keep)*138 ; weight w = keep + (1-keep)/59
        tcol = tp.tile([B_CORE, 196], f32, tag="tcol")
        nc.vector.tensor_tensor(tcol[:], ranks[:], keep[:], op=AL.mult)
        nc.vector.scalar_tensor_tensor(tcol[:], keep[:], -float(N1 - 1), tcol[:],
                                       op0=AL.mult, op1=AL.add)
        nc.vector.tensor_scalar(tcol[:], tcol[:], float(N1 - 1), None, op0=AL.add)
        wcol = tp.tile([B_CORE, 196], f32, tag="wcol")
        nc.vector.tensor_scalar(wcol[:], keep[:], float((N_DROP - 1) / N_DROP),
                                1.0 / N_DROP, op0=AL.mult, op1=AL.add)
        tT = [tp.tile([128, B_CORE], f32, tag=f"tT{i}", name=f"tT{i}") for i in range(2)]
        wT = [tp.tile([128, B_CORE], f32, tag=f"wT{i}", name=f"wT{i}") for i in range(2)]
        for i in range(2):
            pt = psB()
            nc.tensor.transpose(pt[0:jl[i], 0:B_CORE],
                                tcol[:, i * 128:i * 128 + jl[i]],
                                ident[0:B_CORE, 0:B_CORE])
            nc.vector.tensor_copy(tT[i][0:jl[i], :], pt[0:jl[i], 0:B_CORE])
            pt2 = psB()
            nc.tensor.transpose(pt2[0:jl[i], 0:B_CORE],
                                wcol[:, i * 128:i * 128 + jl[i]],
                                ident[0:B_CORE, 0:B_CORE])
            nc.vector.tensor_copy(wT[i][0:jl[i], :], pt2[0:jl[i], 0:B_CORE])

        # per pair: transpose old x (img tokens only, cls-skipped so chunks
        # align with P), cls copies, then one-hot gather matmul, in place.
        for p in range(PAIRS):
            xa = xt[p]
            xtok = {}
            for b in range(2):
                for i in range(2):
                    tlen = jl[i]
                    xk = tp.tile([128, CH * 128], f32r, tag=f"xtok{b}{i}")
                    xtok[(b, i)] = xk
                    for k in range(CH):
                        pt = psB()
                        nc.tensor.transpose(
                            pt[0:tlen, 0:128],
                            xa[:, k * F0 + b * N0 + 1 + i * 128:
                               k * F0 + b * N0 + 1 + i * 128 + tlen].bitcast(f32),
                            ident[:])
                        nc.vector.tensor_copy(xk[0:tlen, k * 128:(k + 1) * 128],
                                              pt[0:tlen, 0:128])
            for b in range(2):
                for k in range(CH):
                    nc.vector.tensor_copy(
                        xa[:, k * F1 + b * N1: k * F1 + b * N1 + 1],
                        xa[:, k * F0 + b * N0: k * F0 + b * N0 + 1])
            for b in range(2):
                img = 2 * p + b
                P = [tp.tile([128, N1 - 1], f32r, tag=f"P{i}", name=f"P{i}") for i in range(2)]
                for i in range(2):
                    nc.vector.tensor_scalar(
                        P[i][0:jl[i], :], iota[0:jl[i], :],
                        tT[i][0:jl[i], img:img + 1], wT[i][0:jl[i], img:img + 1],
                        op0=AL.is_equal, op1=AL.mult)
                for k in range(CH):
                    pg = psC()
                    for i in range(2):
                        nc.tensor.matmul(
                            pg[0:128, 0:N1 - 1],
                            xtok[(b, i)][0:jl[i], k * 128:(k + 1) * 128],
                            P[i][0:jl[i], :],
                            start=(i == 0), stop=(i == 1))
                    nc.vector.tensor_copy(
                        xa[:, k * F1 + b * N1 + 1: k * F1 + b * N1 + N1],
                        pg[0:128, 0:N1 - 1])


# ------------------------------------------------------------------- host
def _host_pack(inputs):
    """Fold LN affines into weights, pre-transpose, pre-extract patches."""
    f = np.float32
    h = np.float16
    inp = {k: np.asarray(v, f) for k, v in inputs.items()}
    out = {}

    imgs = inp['inputs']
    B = imgs.shape[0]
    x = imgs.reshape(B, 3, 14, 16, 14, 16).transpose(0, 2, 4, 1, 3, 5).reshape(B, 196, 768)
    out['patchesT_full'] = np.ascontiguousarray(x.transpose(2, 0, 1).reshape(768, B * 196))

    posC = inp['pos_embed'][0].copy()
    posC[0] += inp['cls_token'][0, 0]
    posC[1:] += inp['patch_b'][None, :]
    out['posCT'] = np.ascontiguousarray(posC.T)

    out['patch_wT'] = np.ascontiguousarray(inp['patch_w'].reshape(C, -1).T)

    qkv_wT = np.empty((DEPTH, C, 3 * C), h)
    qkv_bL = np.empty((DEPTH, 128, 18), f)
    proj_wT = np.empty((DEPTH, C, C), h)
    proj_bL = np.empty((DEPTH, 128, 6), f)
    fc1_wT = np.empty((DEPTH, C, 4 * C), h)
    fc1_bL = np.empty((DEPTH, 128, 24), f)
    fc2_wT = np.empty((DEPTH, 4 * C, C), h)
    fc2_bL = np.empty((DEPTH, 128, 6), f)
    for l in range(DEPTH):
        w1 = inp['qkv_w'][l] * inp['ln1_g'][l][None, :]
        b1 = inp['qkv_b'][l] + inp['qkv_w'][l] @ inp['ln1_b'][l]
        # fold the attention SCALE into the q weights/bias
        w1[:C] *= np.float32(SCALE)
        b1 = b1.copy()
        b1[:C] *= np.float32(SCALE)
        qkv_wT[l] = w1.T.astype(h)
        qkv_bL[l] = b1.reshape(18, 128).T
        proj_wT[l] = inp['proj_w'][l].T.astype(h)
        proj_bL[l] = inp['proj_b'][l].reshape(6, 128).T
        wf1 = inp['fc1_w'][l] * inp['ln2_g'][l][None, :]
        bf1 = inp['fc1_b'][l] + inp['fc1_w'][l] @ inp['ln2_b'][l]
        fc1_wT[l] = wf1.T.astype(h)
        fc1_bL[l] = bf1.reshape(24, 128).T
        fc2_wT[l] = inp['fc2_w'][l].T.astype(h)
        fc2_bL[l] = inp['fc2_b'][l].reshape(6, 128).T
    out.update(qkv_wT=qkv_wT, qkv_bL=qkv_bL, proj_wT=proj_wT, proj_bL=proj_bL,
               fc1_wT=fc1_wT, fc1_bL=fc1_bL, fc2_wT=fc2_wT, fc2_bL=fc2_bL)

    hw = inp['head_w'] * inp['norm_g'][None, :]
    hb = inp['head_b'] + inp['head_w'] @ inp['norm_b']
    out['headT'] = np.ascontiguousarray(hw.T.astype(h))
    out['head_bL'] = np.ascontiguousarray(hb.reshape(NCLS, 1))

    out['ident'] = np.eye(128, dtype=f)
    out['ones'] = np.ones((128, 128), h)
    out['invC'] = np.full((128, 128), 1.0 / C, f)
    out['invC16'] = np.full((128, 128), 1.0 / C, h)
    out['iota'] = np.tile(np.arange(1, N1, dtype=f), (128, 1))
    out['LT'] = (np.arange(196)[:, None] <= np.arange(196)[None, :]).astype(f)
    return out


_BUILT = None


def kernel(**inputs):
    global _BUILT
    host = _host_pack(inputs)
    if _BUILT is None:
        nc = build_nc()
        split_excess_waits(nc)
        _BUILT = nc
    nc = _BUILT

    shared_keys = ['posCT', 'patch_wT', 'qkv_wT', 'qkv_bL', 'proj_wT', 'proj_bL',
                   'fc1_wT', 'fc1_bL', 'fc2_wT', 'fc2_bL', 'headT', 'head_bL',
                   'ident', 'ones', 'invC', 'invC16', 'iota', 'LT']
    in_maps = []
    for c in range(NCORES):
        m = {k: host[k] for k in shared_keys}
        m['patchesT'] = np.ascontiguousarray(
            host['patchesT_full'][:, c * B_CORE * 196:(c + 1) * B_CORE * 196])
        in_maps.append(m)

    trace = bool(os.environ.get("BASS_VIT_TRACE"))
    res = run_bass_kernel_spmd(nc, in_maps, core_ids=list(range(NCORES)), trace=trace)
    if trace:
        print(f"HW exec time: {res.exec_time_ns} ns (mean {res.mean_exec_time_ns})")
        kernel.last_exec_time_ns = res.exec_time_ns

    out = np.concatenate([res.results[c]["logitsT"].T for c in range(NCORES)],
                         axis=0).astype(np.float32)
    if os.environ.get("BASS_VIT_DEBUG_LAYER", ""):
        kernel.last_dbg = [res.results[c].get("dbg") for c in range(NCORES)]
        kernel.last_dbgp = [res.results[c].get("dbgp") for c in range(NCORES)]
    return out


# revision 26
# speedup vs baseline: 1.1025x; 1.0955x over previous
"""CertViT (ViT-Base + layer-3 token pruning) forward pass on 8 Trainium2 cores.

Data parallel: 8 images per core, processed as 4 image-pairs so dense matmul
free dims (394 / 278) stay >= 256. Activations live in channel-partition
layout x^T [768 -> 6x128 chunks, tokens]; the residual stream x stays fp32,
everything fed to the PE (post-LN activations, q/k/v, exp weights, weights)
is fp16 so small-free-dim attention matmuls run at 1 cycle/row and DVE ops
get the 2x/4x modes. Attention processes heads in even/odd pairs: QK is
row-tiled (contraction 64: even head rows 0:63, odd 64:127), the softmax
denominator and AV are col-tiled (output partitions 0:63 / 64:127), so head
pairs run concurrently in the PE array and odd heads no longer need a
partition-shift DMA. Both images of a pair share one QK PSUM bank
([keys, img0 queries | img1 queries]), halving exp instruction count and AV
streamed columns. All reciprocals (softmax denom, LN rsqrt, uncertainty) are
computed as Exp(-k*Ln(x)) on ScalarE -- one activation-table set shared with
the attention Exp, nothing iterative on DVE. LayerNorm affine params are
folded into the following matmul weights on the host. Top-k pruning uses
max8/match_replace for the drop mask, a triangular-matmul cumsum for ranks,
and a one-hot permutation matmul for the gather.
"""

import os
import sys

import numpy as np

for _p in ('/opt/trn_rl_repo', '/root/.axon_site/_ro/trn_rl_repo'):
    if os.path.isdir(_p) and _p not in sys.path:
        sys.path.append(_p)

import concourse.bass as bass
import concourse.mybir as mybir
from concourse.tile import TileContext
from concourse.bass_utils import run_bass_kernel_spmd
from concourse.alu_op_type import AluOpType as AL

dt = mybir.dt
AF = mybir.ActivationFunctionType

# ---------------------------------------------------------------- config
NCORES = 8
B_CORE = 8            # images per core
PAIRS = B_CORE // 2
C = 768
CH = C // 128          # 6 channel chunks
HD = 12                # heads
HP = HD // 2           # head pairs
D = 64                 # head dim
SCALE = D ** -0.5
DEPTH = 12
SEL = 3                # pruning layer
N0 = 197               # tokens before pruning
K_KEEP = 137           # int(197*0.7)
N_DROP = N0 - 1 - K_KEEP   # 59
N1 = K_KEEP + 2        # 139 tokens after pruning
F0 = 2 * N0            # pair free dim, layers 0..3
F1 = 2 * N1            # pair free dim, layers 4..11
EPS = 1e-6
NCLS = 100

# ------------------------------------------------------------- waitfix
# This walrus build accepts at most ONE sem wait per instruction; Tile can
# attach several. Move excess waits onto InstNoOp carriers inserted before.
_wf_counter = [0]


def _wf_carrier(engine, waits):
    _wf_counter[0] += 1
    d = mybir.InstNoOp(name=f"waitfix-{_wf_counter[0]}", ins=[], outs=[])
    d.engine = engine
    d.sync_info = mybir.SyncInfo(on_wait=list(waits), on_update=[])
    return d


def split_excess_waits(nc, max_waits=1):
    nfix = 0
    for f in nc.m.functions:
        for bb in f.blocks:
            insts = list(bb.instructions)
            out = []
            changed = False
            for inst in insts:
                si = inst.sync_info
                waits = list(si.on_wait) if si and si.on_wait else []
                if len(waits) > max_waits:
                    keep, rest = waits[:max_waits], waits[max_waits:]
                    while rest:
                        chunk, rest = rest[:max_waits], rest[max_waits:]
                        out.append(_wf_carrier(inst.engine, chunk))
                    si.on_wait = keep
                    changed = True
                    nfix += 1
                out.append(inst)
            if changed:
                bb.instructions = out
    return nfix


# ----------------------------------------------------------- device kernel
def build_nc():
    nc = bass.Bass()
    f32, f32r, f16 = dt.float32, dt.float32r, dt.float16

    d = {}
    d["patches_d"] = nc.declare_dram_parameter("patchesT", [C, B_CORE * 196], f32r, isOutput=False)
    d["posc_d"] = nc.declare_dram_parameter("posCT", [C, N0], f32, isOutput=False)
    d["pw_d"] = nc.declare_dram_parameter("patch_wT", [C, C], f32r, isOutput=False)
    d["qkvw_d"] = nc.declare_dram_parameter("qkv_wT", [DEPTH, C, 3 * C], f16, isOutput=False)
    d["qkvb_d"] = nc.declare_dram_parameter("qkv_bL", [DEPTH, 128, 18], f32, isOutput=False)
    d["projw_d"] = nc.declare_dram_parameter("proj_wT", [DEPTH, C, C], f16, isOutput=False)
    d["projb_d"] = nc.declare_dram_parameter("proj_bL", [DEPTH, 128, 6], f32, isOutput=False)
    d["fc1w_d"] = nc.declare_dram_parameter("fc1_wT", [DEPTH, C, 4 * C], f16, isOutput=False)
    d["fc1b_d"] = nc.declare_dram_parameter("fc1_bL", [DEPTH, 128, 24], f32, isOutput=False)
    d["fc2w_d"] = nc.declare_dram_parameter("fc2_wT", [DEPTH, 4 * C, C], f16, isOutput=False)
    d["fc2b_d"] = nc.declare_dram_parameter("fc2_bL", [DEPTH, 128, 6], f32, isOutput=False)
    d["headw_d"] = nc.declare_dram_parameter("headT", [C, NCLS], f16, isOutput=False)
    d["headb_d"] = nc.declare_dram_parameter("head_bL", [NCLS, 1], f32, isOutput=False)
    d["ident_d"] = nc.declare_dram_parameter("ident", [128, 128], f32, isOutput=False)
    d["ones_d"] = nc.declare_dram_parameter("ones", [128, 128], f16, isOutput=False)
    d["invc_d"] = nc.declare_dram_parameter("invC", [128, 128], f32r, isOutput=False)
    d["invc16_d"] = nc.declare_dram_parameter("invC16", [128, 128], f16, isOutput=False)
    d["iota_d"] = nc.declare_dram_parameter("iota", [128, N1 - 1], f32, isOutput=False)
    d["lt_d"] = nc.declare_dram_parameter("LT", [196, 196], f32r, isOutput=False)
    d["out_d"] = nc.declare_dram_parameter("logitsT", [NCLS, B_CORE], f32, isOutput=True)

    d["dbg_layer"] = os.environ.get("BASS_VIT_DEBUG_LAYER", "")
    if d["dbg_layer"]:
        d["dbg_d"] = nc.declare_dram_parameter("dbg", [1 + 2 * DEPTH, 128, CH * F0], f32, isOutput=True)
        d["dbgp_d"] = nc.declare_dram_parameter("dbgp", [4, 8, 196], f32, isOutput=True)
    else:
        d["dbg_d"] = None
        d["dbgp_d"] = None

    with TileContext(nc) as tc:
        _build_body(nc, tc, d)
    return nc


def _build_body(nc, tc, d):
    f32, f32r, f16 = dt.float32, dt.float32r, dt.float16
    from contextlib import ExitStack
    es = ExitStack()

    cpool = es.enter_context(tc.tile_pool(name="consts", bufs=1))
    xpool = es.enter_context(tc.tile_pool(name="x", bufs=1))
    ppool = es.enter_context(tc.tile_pool(name="psum", bufs=1, space="PSUM"))
    prpool = es.enter_context(tc.tile_pool(name="prune", bufs=1))
    bpool = es.enter_context(tc.tile_pool(name="bias", bufs=2))

    # constants
    ident = cpool.tile([128, 128], f32, tag="ident")
    ones = cpool.tile([128, 128], f16, tag="ones")
    invc = cpool.tile([128, 128], f32r, tag="invc")
    invc16 = cpool.tile([128, 128], f16, tag="invc16")
    iota = cpool.tile([128, N1 - 1], f32, tag="iota")
    ltt = cpool.tile([128, 2 * 196], f32r, tag="ltt")
    posct = cpool.tile([128, CH * N0], f32, tag="posct")
    eps_t = cpool.tile([128, 1], f32, tag="eps_t")
    nc.vector.memset(eps_t[:], EPS)
    n0_t = cpool.tile([128, 1], f32, tag="n0_t")
    nc.vector.memset(n0_t[:], float(N0))
    nc.sync.dma_start(ident[:], d["ident_d"][:])
    nc.sync.dma_start(ones[:], d["ones_d"][:])
    nc.sync.dma_start(invc[:], d["invc_d"][:])
    nc.sync.dma_start(invc16[:], d["invc16_d"][:])
    nc.sync.dma_start(iota[:], d["iota_d"][:])
    nc.sync.dma_start(ltt[:, 0:196], d["lt_d"][0:128, :])
    nc.sync.dma_start(ltt[0:68, 196:392], d["lt_d"][128:196, :])
    nc.sync.dma_start(posct[:].rearrange("p (k n) -> p k n", k=CH), d["posc_d"].rearrange("(k p) n -> p k n", p=128))

    # PSUM slots: tag 'a' x4 (main accumulations + QK), 'b' x2 (denominator),
    # 'c' x2 (AV / LN meansq) -> 8 banks
    def psA():
        return ppool.tile([128, F0], f32, tag="a", bufs=4, name="psA")

    def psB():
        return ppool.tile([128, F0], f32, tag="b", bufs=2, name="psB")

    def psC():
        return ppool.tile([128, F0], f32, tag="c", bufs=2, name="psC")

    # persistent per-pair residual stream x^T, chunk-major [128, CH*F]
    xt = [xpool.tile([128, CH * F0], f32r, tag=f"x{p}", name=f"x{p}") for p in range(PAIRS)]
    # per-pair uncertainty rows (filled at layer SEL)
    unc = [prpool.tile([1, F0], f32, tag=f"unc{p}", name=f"unc{p}") for p in range(PAIRS)]

    # ------------------------------------------------------------ patch embed
    with tc.tile_pool(name="wpatch", bufs=1) as wp, tc.tile_pool(name="tpatch", bufs=2) as tp:
        pwt = wp.tile([128, CH * C], f32r, tag="pw")
        nc.sync.dma_start(pwt[:].rearrange("p (k n) -> p k n", k=CH), d["pw_d"].rearrange("(k p) n -> p k n", p=128))
        for p in range(PAIRS):
            prt = tp.tile([128, CH * 392], f32r, tag="patches")
            nc.sync.dma_start(
                prt[:].rearrange("p (k n) -> p k n", k=CH),
                d["patches_d"][:, p * 392:(p + 1) * 392].rearrange("(k p) n -> p k n", p=128),
            )
            for co in range(CH):
                ps = psA()
                for k in range(CH):
                    nc.tensor.matmul(
                        ps[:, 0:392],
                        pwt[:, k * C + co * 128: k * C + co * 128 + 128],
                        prt[:, k * 392:(k + 1) * 392],
                        start=(k == 0), stop=(k == CH - 1),
                    )
                for b in range(2):
                    nc.vector.tensor_tensor(
                        xt[p][:, co * F0 + b * N0 + 1: co * F0 + b * N0 + N0],
                        ps[:, b * 196:(b + 1) * 196],
                        posct[:, co * N0 + 1: co * N0 + N0],
                        op=AL.add,
                    )
                    nc.vector.tensor_copy(
                        xt[p][:, co * F0 + b * N0: co * F0 + b * N0 + 1],
                        posct[:, co * N0: co * N0 + 1],
                    )

    def tap(slot, xtile, F):
        if d["dbg_d"] is not None:
            nc.sync.dma_start(d["dbg_d"][slot][:, 0:CH * F], xtile[:, 0:CH * F].bitcast(f32))

    tap(0, xt[0], F0)

    # ------------------------------------------------------------ helpers
    def layernorm(pool, x, F, xh_tag, xh_bufs=1):
        """Standardize x (chunk-major [128, CH*F]) per token -> fp16 tile."""
        xh = pool.tile([128, CH * F], f16, tag=xh_tag, bufs=xh_bufs, name=xh_tag)
        sq = pool.tile([128, CH * F], f16, tag="ln_sq", bufs=2)
        for k in range(CH):
            nc.scalar.activation(
                sq[:, k * F:(k + 1) * F],
                x[:, k * F:(k + 1) * F].bitcast(f32), AF.Square)
        pm = psB()
        ps2 = psC()
        for k in range(CH):
            nc.tensor.matmul(pm[:, 0:F], invc[:], x[:, k * F:(k + 1) * F],
                             start=(k == 0), stop=(k == CH - 1))
        for k in range(CH):
            nc.tensor.matmul(ps2[:, 0:F], invc16[:], sq[:, k * F:(k + 1) * F],
                             start=(k == 0), stop=(k == CH - 1))
        var = pool.tile([128, F], f32, tag="ln_var", bufs=2)
        rstd = pool.tile([128, F], f32, tag="ln_rstd", bufs=2)
        mean = pool.tile([128, F], f32, tag="ln_mean", bufs=2)
        nc.vector.tensor_copy(mean[:], pm[:, 0:F])
        nc.vector.tensor_tensor(var[:], mean[:], mean[:], op=AL.mult)
        nc.vector.tensor_tensor(var[:], ps2[:, 0:F], var[:], op=AL.subtract)
        # rstd = exp(-0.5*ln(var+eps)) = 1/sqrt(var+eps); Ln+Exp share one
        # activation-table set with the attention Exp.
        nc.scalar.activation(rstd[:], var[:], AF.Ln, bias=eps_t[:, 0:1])
        nc.scalar.activation(rstd[:], rstd[:], AF.Exp, scale=-0.5)
        for k in range(CH):
            nc.vector.tensor_tensor(
                var[:], x[:, k * F:(k + 1) * F].bitcast(f32), mean[:], op=AL.subtract)
            nc.vector.tensor_tensor(
                xh[:, k * F:(k + 1) * F], var[:], rstd[:], op=AL.mult)
        return xh

    def load_bias(dram_t, l, cols):
        bt = bpool.tile([128, cols], f32, tag=dram_t.name)
        nc.sync.dma_start(bt[:], dram_t[l])
        return bt

    # ------------------------------------------------------------ layers
    for l in range(DEPTH):
        F = F0 if l <= SEL else F1
        N = N0 if l <= SEL else N1
        mlens = [128, N - 128]

        qkvb = load_bias(d["qkvb_d"], l, 18)
        projb = load_bias(d["projb_d"], l, 6)

        # ---------------- phase A: LN1 + QKV + attention + proj ----------------
        with tc.tile_pool(name="wA", bufs=1) as wA, tc.tile_pool(name="tA", bufs=1) as tA:
            wq = wA.tile([128, CH * 3 * C], f16, tag="wqkv")
            nc.sync.dma_start(wq[:].rearrange("p (k n) -> p k n", k=CH), d["qkvw_d"][l].rearrange("(k p) n -> p k n", p=128))
            wpj = wA.tile([128, CH * C], f16, tag="wproj")
            nc.sync.dma_start(wpj[:].rearrange("p (k n) -> p k n", k=CH), d["projw_d"][l].rearrange("(k p) n -> p k n", p=128))

            xhs = [layernorm(tA, xt[p], F, "ln1", xh_bufs=4) for p in range(PAIRS)]
            for p in range(PAIRS):
                xh = xhs[p]
                qT = tA.tile([128, CH * F], f16, tag="qT", bufs=2, name="qT")
                kT = tA.tile([128, CH * F], f16, tag="kT", bufs=2, name="kT")
                for o in range(12):
                    ps = psA()
                    for k in range(CH):
                        nc.tensor.matmul(
                            ps[:, 0:F],
                            wq[:, k * 3 * C + o * 128: k * 3 * C + o * 128 + 128],
                            xh[:, k * F:(k + 1) * F],
                            start=(k == 0), stop=(k == CH - 1),
                        )
                    oc = o % CH
                    # SCALE is folded into the q weights/bias on the host
                    dst = qT if o < CH else kT
                    nc.vector.tensor_scalar(
                        dst[:, oc * F:(oc + 1) * F], ps[:, 0:F],
                        qkvb[:, o:o + 1], None, op0=AL.add)

                # v in token-partition layout, per image: 2 t-chunks
                vto = [[None, None], [None, None]]
                for b in range(2):
                    for tchunk in range(2):
                        tlen = mlens[tchunk]
                        toff = b * N + tchunk * 128
                        vt = tA.tile([128, C], f16, tag=f"v{b}{tchunk}", bufs=2)
                        vto[b][tchunk] = vt
                        for half in range(2):
                            ps = psA()
                            for k in range(CH):
                                nc.tensor.matmul(
                                    ps[0:tlen, 0:384],
                                    xh[:, k * F + toff: k * F + toff + tlen],
                                    wq[:, k * 3 * C + 2 * C + half * 384:
                                       k * 3 * C + 2 * C + half * 384 + 384],
                                    start=(k == 0), stop=(k == CH - 1),
                                )
                            nc.vector.tensor_copy(
                                vt[0:tlen, half * 384:(half + 1) * 384],
                                ps[0:tlen, 0:384])

                # attention by head pair hp: even head e=0 on rows/out-cols
                # 0:64, odd e=1 on 64:128 (row-tiled QK, col-tiled denom/AV).
                # Pass 1 (all hp): QK + exp; pass 2 (all hp): denom/AV/norm --
                # keeps the PE stream free of exp-latency head-of-line stalls.
                oT = tA.tile([128, CH * F], f16, tag="oT", bufs=2, name="oT")
                ets = {}
                pevs = {}

                def qk_pass(hp):
                    qcol = hp * F
                    et = [[tA.tile([128, F], f16, tag=f"et{e}{t}", bufs=6,
                                   name=f"et{e}{t}") for t in range(2)]
                          for e in range(2)]
                    ets[hp] = et
                    if l == SEL:
                        pevs[hp] = [psB(), psA()]
                    for tchunk in range(2):
                        tlen = mlens[tchunk]
                        toff = tchunk * 128
                        psQK = [psA(), psA()]
                        for b in range(2):
                            for e in range(2):
                                nc.tensor.matmul(
                                    psQK[e][0:tlen, b * N:(b + 1) * N],
                                    kT[e * 64:e * 64 + 64,
                                       qcol + b * N + toff: qcol + b * N + toff + tlen],
                                    qT[e * 64:e * 64 + 64, qcol + b * N: qcol + (b + 1) * N],
                                    start=True, stop=True,
                                )
                        for e in range(2):
                            nc.scalar.activation(
                                et[e][tchunk][0:tlen, 0:F],
                                psQK[e][0:tlen, 0:F], AF.Exp)
                        if l == SEL:
                            for e in range(2):
                                rt = tA.tile([128, F], f16, tag=f"relu{e}", bufs=1)
                                nc.vector.tensor_scalar(
                                    rt[0:tlen, 0:F], psQK[e][0:tlen, 0:F],
                                    0.0, None, op0=AL.max)
                                nc.tensor.matmul(
                                    pevs[hp][e][0:1, 0:F], ones[0:tlen, 0:1],
                                    rt[0:tlen, 0:F],
                                    start=(tchunk == 0), stop=(tchunk == 1),
                                )

                def av_pass(hp):
                    qcol = hp * F
                    et = ets.pop(hp)
                    pden = psB()
                    # one AV bank per image; parities col-tiled on disjoint
                    # partitions (0:64 / 64:128), so their accumulation groups
                    # interleave safely within a bank.
                    pav = [psC(), psC()]
                    for tchunk in range(2):
                        tlen = mlens[tchunk]
                        for e in range(2):
                            nc.tensor.matmul(
                                pden[e * 64:e * 64 + 64, 0:F],
                                ones[0:tlen, 0:64],
                                et[e][tchunk][0:tlen, 0:F],
                                start=(tchunk == 0), stop=(tchunk == 1),
                            )
                        for b in range(2):
                            for e in range(2):
                                nc.tensor.matmul(
                                    pav[b][e * 64:e * 64 + 64, 0:N],
                                    vto[b][tchunk][0:tlen,
                                                   (2 * hp + e) * 64:(2 * hp + e) * 64 + 64],
                                    et[e][tchunk][0:tlen, b * N:(b + 1) * N],
                                    start=(tchunk == 0), stop=(tchunk == 1),
                                )
                    if l == SEL:
                        # unc += 1/(evidence_sum + N) per head
                        for e in range(2):
                            ev1 = tA.tile([1, F], f32, tag="ev1", bufs=2)
                            nc.scalar.activation(
                                ev1[:], pevs[hp][e][0:1, 0:F], AF.Ln,
                                bias=n0_t[0:1, 0:1])
                            nc.scalar.activation(ev1[:], ev1[:], AF.Exp, scale=-1.0)
                            if hp == 0 and e == 0:
                                nc.vector.tensor_copy(unc[p][:], ev1[:])
                            else:
                                nc.vector.tensor_tensor(
                                    unc[p][:], ev1[:], unc[p][:], op=AL.add)
                        pevs.pop(hp)
                    # rsb = 1/denominator via exp(-ln), both parities at once
                    rsb = tA.tile([128, F], f32, tag="rsb", bufs=2)
                    nc.scalar.activation(rsb[:], pden[0:128, 0:F], AF.Ln)
                    nc.scalar.activation(rsb[:], rsb[:], AF.Exp, scale=-1.0)
                    for b in range(2):
                        nc.vector.tensor_tensor(
                            oT[:, qcol + b * N:qcol + (b + 1) * N],
                            pav[b][0:128, 0:N],
                            rsb[:, b * N:(b + 1) * N], op=AL.mult)
                    # v-bias for the whole chunk
                    nc.vector.tensor_scalar(
                        oT[:, qcol:qcol + F], oT[:, qcol:qcol + F],
                        qkvb[:, 12 + hp:13 + hp], None, op0=AL.add)

                if l == SEL:
                    # pev PSUM lifetimes don't allow the two-pass split here
                    for hp in range(HP):
                        qk_pass(hp)
                        av_pass(hp)
                else:
                    for hp in range(HP):
                        qk_pass(hp)
                    for hp in range(HP):
                        av_pass(hp)

                # proj + residual
                for co in range(CH):
                    ps = psA()
                    for k in range(CH):
                        nc.tensor.matmul(
                            ps[:, 0:F],
                            wpj[:, k * C + co * 128: k * C + co * 128 + 128],
                            oT[:, k * F:(k + 1) * F],
                            start=(k == 0), stop=(k == CH - 1),
                        )
                    nc.vector.scalar_tensor_tensor(
                        xt[p][:, co * F:(co + 1) * F],
                        ps[:, 0:F], projb[:, co:co + 1],
                        xt[p][:, co * F:(co + 1) * F].bitcast(f32),
                        op0=AL.add, op1=AL.add)

        tap(1 + 2 * l, xt[0], F)

        # ---------------- pruning (after layer-SEL attention residual) --------
        if l == SEL:
            _prune(nc, tc, xt, unc, ident, ltt, iota, psB, psC, d)

        F = F0 if l < SEL else F1

        fc1b = load_bias(d["fc1b_d"], l, 24)
        fc2b = load_bias(d["fc2b_d"], l, 6)

        # ---------------- phase B: LN2 + MLP in 4 quarters ---------------------
        with tc.tile_pool(name="wB", bufs=1) as wB, tc.tile_pool(name="tB", bufs=1) as tB:
            xh2 = [layernorm(tB, xt[p], F, f"ln2_{p}") for p in range(PAIRS)]
            h1 = [tB.tile([128, CH * F], f16, tag=f"h1_{p}", name=f"h1_{p}") for p in range(PAIRS)]
            for q in range(4):
                w1 = wB.tile([128, CH * C], f16, tag="wfc1", bufs=2)
                nc.sync.dma_start(
                    w1[:].rearrange("p (k n) -> p k n", k=CH),
                    d["fc1w_d"][l][:, q * C:(q + 1) * C].rearrange("(k p) n -> p k n", p=128))
                w2 = wB.tile([128, CH * C], f16, tag="wfc2", bufs=2)
                nc.sync.dma_start(
                    w2[:].rearrange("p (k n) -> p k n", k=CH),
                    d["fc2w_d"][l][q * C:(q + 1) * C, :].rearrange("(k p) n -> p k n", p=128))
                for p in range(PAIRS):
                    for co in range(CH):
                        ps = psA()
                        for k in range(CH):
                            nc.tensor.matmul(
                                ps[:, 0:F],
                                w1[:, k * C + co * 128: k * C + co * 128 + 128],
                                xh2[p][:, k * F:(k + 1) * F],
                                start=(k == 0), stop=(k == CH - 1),
                            )
                        nc.scalar.activation(
                            h1[p][:, co * F:(co + 1) * F], ps[:, 0:F],
                            AF.Gelu, bias=fc1b[:, q * CH + co:q * CH + co + 1])
                    for co in range(CH):
                        ps = psA()
                        for k in range(CH):
                            nc.tensor.matmul(
                                ps[:, 0:F],
                                w2[:, k * C + co * 128: k * C + co * 128 + 128],
                                h1[p][:, k * F:(k + 1) * F],
                                start=(k == 0), stop=(k == CH - 1),
                            )
                        if q == 0:
                            nc.vector.scalar_tensor_tensor(
                                xt[p][:, co * F:(co + 1) * F],
                                ps[:, 0:F], fc2b[:, co:co + 1],
                                xt[p][:, co * F:(co + 1) * F].bitcast(f32),
                                op0=AL.add, op1=AL.add)
                        else:
                            nc.vector.tensor_tensor(
                                xt[p][:, co * F:(co + 1) * F],
                                ps[:, 0:F],
                                xt[p][:, co * F:(co + 1) * F].bitcast(f32),
                                op=AL.add)
        tap(2 + 2 * l, xt[0], F)

    # ------------------------------------------------------------ head
    with tc.tile_pool(name="whead", bufs=1) as wh, tc.tile_pool(name="thead", bufs=1) as th:
        clsT = th.tile([128, CH * B_CORE], f32r, tag="clsT")
        for p in range(PAIRS):
            for b in range(2):
                for k in range(CH):
                    nc.vector.tensor_copy(
                        clsT[:, k * B_CORE + 2 * p + b: k * B_CORE + 2 * p + b + 1],
                        xt[p][:, k * F1 + b * N1: k * F1 + b * N1 + 1])
        xhc = layernorm(th, clsT, B_CORE, "lnf")
        hw = wh.tile([128, CH * NCLS], f16, tag="hw")
        nc.sync.dma_start(hw[:].rearrange("p (k n) -> p k n", k=CH), d["headw_d"].rearrange("(k p) n -> p k n", p=128))
        hb = wh.tile([NCLS, 1], f32, tag="hb")
        nc.sync.dma_start(hb[:], d["headb_d"][:])
        ps = psC()
        for k in range(CH):
            nc.tensor.matmul(
                ps[0:NCLS, 0:B_CORE],
                hw[:, k * NCLS:(k + 1) * NCLS],
                xhc[:, k * B_CORE:(k + 1) * B_CORE],
                start=(k == 0), stop=(k == CH - 1),
            )
        lt = th.tile([NCLS, B_CORE], f32, tag="logits")
        nc.vector.tensor_scalar(lt[:], ps[0:NCLS, 0:B_CORE], hb[:, 0:1], None, op0=AL.add)
        nc.sync.dma_start(d["out_d"][:], lt[:])

    es.close()


def _prune(nc, tc, xt, unc, ident, ltt, iota, psB, psC, d):
    """Keep the K_KEEP lowest-uncertainty image tokens (drop the N_DROP
    highest), append mean of dropped; rewrite x in-place to [128, CH*F1]."""
    f32, f32r = dt.float32, dt.float32r
    jl = [128, 68]          # img-token chunk lengths (196 = 128 + 68)
    with tc.tile_pool(name="tprune", bufs=1) as tp:
        U = tp.tile([B_CORE, 196], f32, tag="U")
        for p in range(PAIRS):
            for b in range(2):
                # DVE writes must start at a 32-aligned partition; use DMA
                nc.sync.dma_start(
                    U[2 * p + b:2 * p + b + 1, :],
                    unc[p][:, b * N0 + 1:(b + 1) * N0])
        # drop mask: top-N_DROP largest per row (unc ~ 1, min_val 0 is safe;
        # mask threshold min(.,1) needs kept residuals >= 1?  values here are
        # sums of 12 reciprocals in (0,1): ~0.6..1.2 -- scale first to be safe.
        nc.vector.tensor_scalar(U[:], U[:], 100.0, None, op0=AL.mult)
        work = tp.tile([B_CORE, 196], f32, tag="work")
        mx = tp.tile([B_CORE, 8], f32, tag="mx")
        cur = U
        for k_on in range(0, N_DROP, 8):
            nfind = min(k_on + 8, N_DROP) - k_on
            nc.vector.max(out=mx[:], in_=cur[:])
            if nfind < 8:
                nc.vector.memset(mx[:, nfind:], 0.0)
            nc.vector.match_replace(out=work[:], in_to_replace=mx[:],
                                    in_values=cur[:], imm_value=0.0)
            cur = work
        nc.vector.tensor_sub(work[:], U[:], work[:])
        nc.vector.tensor_scalar_min(work[:], work[:], 1.0)   # drop mask {0,1}
        keep = tp.tile([B_CORE, 196], f32, tag="keep")
        nc.vector.tensor_scalar(keep[:], work[:], -1.0, 1.0, op0=AL.mult, op1=AL.add)
        if d.get("dbgp_d") is not None:
            nc.sync.dma_start(d["dbgp_d"][0][0:8, :], U[:])
            nc.sync.dma_start(d["dbgp_d"][1][0:8, :], keep[:])

        # keepT chunks via PE transpose
        keepT = [tp.tile([128, B_CORE], f32r, tag=f"keepT{i}", name=f"keepT{i}") for i in range(2)]
        for i in range(2):
            pt = psB()
            nc.tensor.transpose(pt[0:jl[i], 0:B_CORE],
                                keep[:, i * 128:i * 128 + jl[i]],
                                ident[0:B_CORE, 0:B_CORE])
            nc.vector.tensor_copy(keepT[i][0:jl[i], :], pt[0:jl[i], 0:B_CORE])
        # ranks = inclusive cumsum of keep via lower-triangular ones matmul
        prk = psC()
        for i in range(2):
            nc.tensor.matmul(
                prk[0:B_CORE, 0:196], keepT[i][0:jl[i], :],
                ltt[0:jl[i], i * 196:(i + 1) * 196],
                start=(i == 0), stop=(i == 1))
        ranks = tp.tile([B_CORE, 196], f32, tag="ranks")
        nc.vector.tensor_copy(ranks[:], prk[0:B_CORE, 0:196])
        if d.get("dbgp_d") is not None:
            nc.sync.dma_start(d["dbgp_d"][2][0:8, :], ranks[:])
        # target col t = keep*rank + (1-keep)*138 ; weight w = keep + (1-keep)/59
        tcol = tp.tile([B_CORE, 196], f32, tag="tcol")
        nc.vector.tensor_tensor(tcol[:], ranks[:], keep[:], op=AL.mult)
        nc.vector.scalar_tensor_tensor(tcol[:], keep[:], -float(N1 - 1), tcol[:],
                                       op0=AL.mult, op1=AL.add)
        nc.vector.tensor_scalar(tcol[:], tcol[:], float(N1 - 1), None, op0=AL.add)
        wcol = tp.tile([B_CORE, 196], f32, tag="wcol")
        nc.vector.tensor_scalar(wcol[:], keep[:], float((N_DROP - 1) / N_DROP),
                                1.0 / N_DROP, op0=AL.mult, op1=AL.add)
        tT = [tp.tile([128, B_CORE], f32, tag=f"tT{i}", name=f"tT{i}") for i in range(2)]
        wT = [tp.tile([128, B_CORE], f32, tag=f"wT{i}", name=f"wT{i}") for i in range(2)]
        for i in range(2):
            pt = psB()
            nc.tensor.transpose(pt[0:jl[i], 0:B_CORE],
                                tcol[:, i * 128:i * 128 + jl[i]],
                                ident[0:B_CORE, 0:B_CORE])
            nc.vector.tensor_copy(tT[i][0:jl[i], :], pt[0:jl[i], 0:B_CORE])
            pt2 = psB()
            nc.tensor.transpose(pt2[0:jl[i], 0:B_CORE],
                                wcol[:, i * 128:i * 128 + jl[i]],
                                ident[0:B_CORE, 0:B_CORE])
            nc.vector.tensor_copy(wT[i][0:jl[i], :], pt2[0:jl[i], 0:B_CORE])

        # per pair: transpose old x (img tokens only, cls-skipped so chunks
        # align with P), cls copies, then one-hot gather matmul, in place.
        for p in range(PAIRS):
            xa = xt[p]
            xtok = {}
            for b in range(2):
                for i in range(2):
                    tlen = jl[i]
                    xk = tp.tile([128, CH * 128], f32r, tag=f"xtok{b}{i}")
                    xtok[(b, i)] = xk
                    for k in range(CH):
                        pt = psB()
                        nc.tensor.transpose(
                            pt[0:tlen, 0:128],
                            xa[:, k * F0 + b * N0 + 1 + i * 128:
                               k * F0 + b * N0 + 1 + i * 128 + tlen].bitcast(f32),
                            ident[:])
                        nc.vector.tensor_copy(xk[0:tlen, k * 128:(k + 1) * 128],
                                              pt[0:tlen, 0:128])
            for b in range(2):
                for k in range(CH):
                    nc.vector.tensor_copy(
                        xa[:, k * F1 + b * N1: k * F1 + b * N1 + 1],
                        xa[:, k * F0 + b * N0: k * F0 + b * N0 + 1])
            for b in range(2):
                img = 2 * p + b
                P = [tp.tile([128, N1 - 1], f32r, tag=f"P{i}", name=f"P{i}") for i in range(2)]
                for i in range(2):
                    nc.vector.tensor_scalar(
                        P[i][0:jl[i], :], iota[0:jl[i], :],
                        tT[i][0:jl[i], img:img + 1], wT[i][0:jl[i], img:img + 1],
                        op0=AL.is_equal, op1=AL.mult)
                for k in range(CH):
                    pg = psC()
                    for i in range(2):
                        nc.tensor.matmul(
                            pg[0:128, 0:N1 - 1],
                            xtok[(b, i)][0:jl[i], k * 128:(k + 1) * 128],
                            P[i][0:jl[i], :],
                            start=(i == 0), stop=(i == 1))
                    nc.vector.tensor_copy(
                        xa[:, k * F1 + b * N1 + 1: k * F1 + b * N1 + N1],
                        pg[0:128, 0:N1 - 1])


# ------------------------------------------------------------------- host
def _host_pack(inputs):
    """Fold LN affines into weights, pre-transpose, pre-extract patches."""
    f = np.float32
    h = np.float16
    inp = {k: np.asarray(v, f) for k, v in inputs.items()}
    out = {}

    imgs = inp['inputs']
    B = imgs.shape[0]
    x = imgs.reshape(B, 3, 14, 16, 14, 16).transpose(0, 2, 4, 1, 3, 5).reshape(B, 196, 768)
    out['patchesT_full'] = np.ascontiguousarray(x.transpose(2, 0, 1).reshape(768, B * 196))

    posC = inp['pos_embed'][0].copy()
    posC[0] += inp['cls_token'][0, 0]
    posC[1:] += inp['patch_b'][None, :]
    out['posCT'] = np.ascontiguousarray(posC.T)

    out['patch_wT'] = np.ascontiguousarray(inp['patch_w'].reshape(C, -1).T)

    qkv_wT = np.empty((DEPTH, C, 3 * C), h)
    qkv_bL = np.empty((DEPTH, 128, 18), f)
    proj_wT = np.empty((DEPTH, C, C), h)
    proj_bL = np.empty((DEPTH, 128, 6), f)
    fc1_wT = np.empty((DEPTH, C, 4 * C), h)
    fc1_bL = np.empty((DEPTH, 128, 24), f)
    fc2_wT = np.empty((DEPTH, 4 * C, C), h)
    fc2_bL = np.empty((DEPTH, 128, 6), f)
    for l in range(DEPTH):
        w1 = inp['qkv_w'][l] * inp['ln1_g'][l][None, :]
        b1 = inp['qkv_b'][l] + inp['qkv_w'][l] @ inp['ln1_b'][l]
        # fold the attention SCALE into the q weights/bias
        w1[:C] *= np.float32(SCALE)
        b1 = b1.copy()
        b1[:C] *= np.float32(SCALE)
        qkv_wT[l] = w1.T.astype(h)
        qkv_bL[l] = b1.reshape(18, 128).T
        proj_wT[l] = inp['proj_w'][l].T.astype(h)
        proj_bL[l] = inp['proj_b'][l].reshape(6, 128).T
        wf1 = inp['fc1_w'][l] * inp['ln2_g'][l][None, :]
        bf1 = inp['fc1_b'][l] + inp['fc1_w'][l] @ inp['ln2_b'][l]
        fc1_wT[l] = wf1.T.astype(h)
        fc1_bL[l] = bf1.reshape(24, 128).T
        fc2_wT[l] = inp['fc2_w'][l].T.astype(h)
        fc2_bL[l] = inp['fc2_b'][l].reshape(6, 128).T
    out.update(qkv_wT=qkv_wT, qkv_bL=qkv_bL, proj_wT=proj_wT, proj_bL=proj_bL,
               fc1_wT=fc1_wT, fc1_bL=fc1_bL, fc2_wT=fc2_wT, fc2_bL=fc2_bL)

    hw = inp['head_w'] * inp['norm_g'][None, :]
    hb = inp['head_b'] + inp['head_w'] @ inp['norm_b']
    out['headT'] = np.ascontiguousarray(hw.T.astype(h))
    out['head_bL'] = np.ascontiguousarray(hb.reshape(NCLS, 1))

    out['ident'] = np.eye(128, dtype=f)
    out['ones'] = np.ones((128, 128), h)
    out['invC'] = np.full((128, 128), 1.0 / C, f)
    out['invC16'] = np.full((128, 128), 1.0 / C, h)
    out['iota'] = np.tile(np.arange(1, N1, dtype=f), (128, 1))
    out['LT'] = (np.arange(196)[:, None] <= np.arange(196)[None, :]).astype(f)
    return out


_BUILT = None


def kernel(**inputs):
    global _BUILT
    host = _host_pack(inputs)
    if _BUILT is None:
        nc = build_nc()
        split_excess_waits(nc)
        _BUILT = nc
    nc = _BUILT

    shared_keys = ['posCT', 'patch_wT', 'qkv_wT', 'qkv_bL', 'proj_wT', 'proj_bL',
                   'fc1_wT', 'fc1_bL', 'fc2_wT', 'fc2_bL', 'headT', 'head_bL',
                   'ident', 'ones', 'invC', 'invC16', 'iota', 'LT']
    in_maps = []
    for c in range(NCORES):
        m = {k: host[k] for k in shared_keys}
        m['patchesT'] = np.ascontiguousarray(
            host['patchesT_full'][:, c * B_CORE * 196:(c + 1) * B_CORE * 196])
        in_maps.append(m)

    trace = bool(os.environ.get("BASS_VIT_TRACE"))
    res = run_bass_kernel_spmd(nc, in_maps, core_ids=list(range(NCORES)), trace=trace)
    if trace:
        print(f"HW exec time: {res.exec_time_ns} ns (mean {res.mean_exec_time_ns})")
        kernel.last_exec_time_ns = res.exec_time_ns

    out = np.concatenate([res.results[c]["logitsT"].T for c in range(NCORES)],
                         axis=0).astype(np.float32)
    if os.environ.get("BASS_VIT_DEBUG_LAYER", ""):
        kernel.last_dbg = [res.results[c].get("dbg") for c in range(NCORES)]
        kernel.last_dbgp = [res.results[c].get("dbgp") for c in range(NCORES)]
    return out
